# revision 39
# baseline (speedup 1.0000x reference)
"""Trainium2 Bass kernel for nn_DepthsepCCBlock (dense_cnn).

Strategy: 8-way shard over (batch=2) x (H/4 blocks of 32 rows). Each core
computes its 32 output rows end-to-end. The two training-mode BatchNorms
become sync-BN via two tiny (128x4 fp32) AllReduces, both fully overlapped
with TensorEngine work. The dominant 128->2304 3x3 convs run in fp8 e4m3
DoubleRow mode (two taps contracted per PE pass: 9 taps in 5 passes, 1.8x).
Precision is preserved by mean-removal: the conv input is delta = h - c
(c = per-channel mean, pads become -c automatically since h pads are 0) and
the exact-fp32 tap-sum-of-weights times c is folded back in via a per-group
bias computed on device (18 tiny matmuls). This is cell-wise exact for any
c, so only the fp8 quantization of delta and of the weights remains as
error. All other tensors run in fp16 (same PE/DVE speed as bf16, 8x less
noise). The per-pixel dynamic depthwise 3x3 conv runs on the VectorEngine
as 17 tensor-tensor ops per tile, fused with the producing conv. Halo rows
are recomputed from host-sliced zero-padded shards (no halo exchange);
image-boundary BN-bias artifacts are killed with per-core edge-row masks
supplied as data so every core runs one identical NEFF.
"""
import sys
import types
import numpy as np
import ml_dtypes

if "/opt/trn_rl_repo" not in sys.path:
    sys.path.insert(0, "/opt/trn_rl_repo")

F16NP = np.float16
F8NP = ml_dtypes.float8_e4m3

N, C, H, W = 2, 256, 128, 128
SNC, NH = 35, 128
EPS = 1e-5
N_CORES = 8
RB = 32                      # output rows per shard
CNT = float(N * H * W)       # BN reduction count per channel
WSC = 32.0                   # fp8 weight scale

ROWS_B = 34                  # dx_mid rows (s-1 .. e)
ROWS_C = 32                  # output rows (s .. e-1)

# fp8 tap allocation per gw conv: gw1 taps 0-5 via 3 DoubleRow pairs (taps
# 6,7,8 stay fp16), gw2 taps 0-7 via 4 pairs (tap 8 fp16). Chosen from the
# precision sim: (6,8) taps fp8 -> rel err ~1.6e-2 vs 1.84e-2 for (9,9),
# at the cost of one extra PE pass on gw1 only.
GW1_PAIRS = [(0, 1), (2, 3), (4, 5)]
GW1_F16 = [6, 7, 8]
GW2_PAIRS = [(0, 1), (2, 3), (4, 5), (6, 7)]
GW2_F16 = [8]


def _row_tiles(nrows):
    out = []
    r = 0
    while r < nrows:
        nr = min(4, nrows - r)
        out.append((r, nr))
        r += nr
    return out


_CACHE = {}


def _build(level=6):
    from contextlib import ExitStack
    import concourse.tile as tile
    from concourse import bacc, mybir
    from concourse.ap import AP as APc

    F32 = mybir.dt.float32
    F16 = mybir.dt.float16
    F8 = mybir.dt.float8e4
    AF = mybir.ActivationFunctionType
    OP = mybir.AluOpType
    DR = mybir.MatmulPerfMode.DoubleRow

    nc = bacc.Bacc("TRN2", target_bir_lowering=False, debug=False,
                   num_devices=N_CORES)

    x_d = nc.dram_tensor("x_bf", [128, 2, 36, 128], F16, kind="ExternalInput")
    xr_d = nc.dram_tensor("x_res", [128, 2, 32, 128], F32, kind="ExternalInput")
    seg_d = nc.dram_tensor("seg3", [105, 36, 130], F16, kind="ExternalInput")
    w1_d = nc.dram_tensor("w1l", [105, 3, 4, 128], F16, kind="ExternalInput")
    gw1_d = nc.dram_tensor("gw1l", [128, 6, 18, 128], F8, kind="ExternalInput")
    gw2_d = nc.dram_tensor("gw2l", [128, 8, 18, 128], F8, kind="ExternalInput")
    gw1f_d = nc.dram_tensor("gw1f", [128, 3, 18, 128], F16, kind="ExternalInput")
    gw2f_d = nc.dram_tensor("gw2f", [128, 1, 18, 128], F16, kind="ExternalInput")
    ws_d = nc.dram_tensor("wsuml", [128, 2, 18, 128], F16, kind="ExternalInput")
    wse_d = nc.dram_tensor("wssel", [128, 2, 2, 128], F16, kind="ExternalInput")
    se1_d = nc.dram_tensor("se1l", [128, 9, 2, 128], F8, kind="ExternalInput")
    se2_d = nc.dram_tensor("se2l", [128, 9, 2, 128], F8, kind="ExternalInput")
    c1_d = nc.dram_tensor("c1l", [128, 2, 2, 128], F16, kind="ExternalInput")
    c3_d = nc.dram_tensor("c3l", [128, 2, 2, 128], F16, kind="ExternalInput")
    bv_d = nc.dram_tensor("bvec", [128, 58], F32, kind="ExternalInput")
    out_d = nc.dram_tensor("out", [128, 2, 32, 128], F32, kind="ExternalOutput")

    # bvec columns
    B1C = 0          # 4: first-conv biases (h1, h2, h1se, h2se order: gw1,gw2,se1,se2)
    GB1 = 4          # 18: gw1_b2 per (t,half)
    GB2 = 22         # 18: gw2_b2
    SB1 = 40         # 2: se1_b2 ; SB2=42 ; C1B=44 ; C3B=46
    SB2, C1B, C3B = 42, 44, 46
    BN1G, BN1B, BN2G, BN2B = 48, 50, 52, 54
    EMSK = 56        # 2: top,bot edge masks

    with tile.TileContext(nc) as tc:
        with ExitStack() as ctx:
            static = ctx.enter_context(tc.tile_pool(name="static", bufs=1))
            cwtp = ctx.enter_context(tc.tile_pool(name="cwtp", bufs=2))
            workp = ctx.enter_context(tc.tile_pool(name="work", bufs=2))
            dramp = ctx.enter_context(tc.tile_pool(name="dramp", bufs=1, space="DRAM"))
            ph = ctx.enter_context(tc.tile_pool(name="ph", bufs=2, space="PSUM"))
            pcw = ctx.enter_context(tc.tile_pool(name="pcw", bufs=4, space="PSUM"))
            pu = ctx.enter_context(tc.tile_pool(name="pu", bufs=2, space="PSUM"))

            # ---- static SBUF tiles ----
            xsb = static.tile([128, 2, 36, 128], F16, tag="wbig2", name="xsb")
            seg = static.tile([105, 36, 130], F16, tag="segc", name="segsb")
            w1 = static.tile([105, 3, 4, 128], F16, tag="w1c", name="w1sb")
            gw1 = static.tile([128, 6, 18, 128], F8, tag="gwA", name="gw1sb")
            gw1f = static.tile([128, 3, 18, 128], F16, tag="gwF", name="gw1fsb")
            ws = static.tile([128, 2, 18, 128], F16, tag="wsc", name="wssb")
            wse = static.tile([128, 2, 2, 128], F16, tag="wsec", name="wsesb")
            se1 = static.tile([128, 9, 2, 128], F8, tag="seC", name="se1sb")
            dq1se = static.tile([128, 36, 130], F8, tag="dqS", name="dq1sesb")
            bt1se = static.tile([128, 2], F32, tag="btsec", name="bt1sesb")
            bt2se = static.tile([128, 2], F32, tag="btse2c", name="bt2sesb")
            c1 = static.tile([128, 2, 2, 128], F16, tag="c1c", name="c1sb")
            c3 = static.tile([128, 2, 2, 128], F16, tag="c3c", name="c3sb")
            bv = static.tile([128, 58], F32, tag="bvc", name="bvsb")
            h1 = static.tile([128, 36, 130], F16, tag="hB", name="h1sb")
            h1se = static.tile([128, 36, 130], F16, tag="hA", name="h1sesb")
            dq1 = static.tile([128, 36, 130], F8, tag="dqA", name="dq1sb")
            xbn = static.tile([128, 2, 36, 130], F16, tag="xbnc", name="xbnsb")
            sw1 = static.tile([128, 2, 34, 128], F16, tag="swc", name="sw1sb")
            dxm = static.tile([128, 2, 34, 130], F16, tag="dxmc", name="dxmsb")
            st1 = static.tile([128, 22], F32, tag="st1c", name="st1sb")
            st2 = static.tile([128, 22], F32, tag="st2c", name="st2sb")
            cv = static.tile([128, 4], F16, tag="cvc", name="cvsb")
            cv32 = static.tile([128, 4], F32, tag="cv32c", name="cv32sb")
            bt1 = static.tile([128, 18], F32, tag="bt1c", name="bt1sb")
            bt2 = static.tile([128, 18], F32, tag="bt2c", name="bt2sb")

            cc1i = dramp.tile([128, 4], F32, name="cc1i")
            cc1o = dramp.tile([128, 4], F32, addr_space="Shared", name="cc1o")
            cc2i = dramp.tile([128, 4], F32, name="cc2i")
            cc2o = dramp.tile([128, 4], F32, addr_space="Shared", name="cc2o")

            sync, ve, act, gp, te = nc.sync, nc.vector, nc.scalar, nc.gpsimd, nc.tensor

            # ---- input DMAs. One queue, critical-first: the first conv needs
            # only seg/w1/bv, and a single in-order queue guarantees the big
            # loads don't steal HBM bandwidth from them (a multi-queue spread
            # measured 23us slower to first matmul).
            sync.dma_start(seg[:], seg_d.ap())
            sync.dma_start(w1[:], w1_d.ap())
            sync.dma_start(bv[:], bv_d.ap())
            sync.dma_start(xsb[:], x_d.ap())
            sync.dma_start(se1[:], se1_d.ap())
            sync.dma_start(ws[:], ws_d.ap())
            sync.dma_start(wse[:], wse_d.ap())
            sync.dma_start(gw1[:], gw1_d.ap())
            sync.dma_start(gw1f[:], gw1f_d.ap())
            sync.dma_start(c1[:], c1_d.ap())
            sync.dma_start(c3[:], c3_d.ap())

            # ---- zero pad cells. Every row and all interior columns of these
            # buffers get written by drains/affines before any read, so only
            # the two pad columns need zeroing.
            def zero_pads(buf):
                if len(buf.shape) == 4:
                    ve.memset(buf[:, :, :, 0], 0.0)
                    ve.memset(buf[:, :, :, 129], 0.0)
                else:
                    ve.memset(buf[:, :, 0], 0.0)
                    ve.memset(buf[:, :, 129], 0.0)

            for buf in (h1, h1se, xbn, dxm):
                zero_pads(buf)

            def bn1_stats():
                # Emitted after the h convs: ScalarE is the drain engine for
                # the first-conv PSUMs, and 15us of stats passes up front
                # stalls the PE behind the 2-bank ph pool.
                scrA = cwtp.tile([128, 32, 128], F16, tag="cwt", name="scrA")
                for hh in range(2):
                    act.activation(scrA[:], xsb[:, hh, 2:34, :], AF.Identity,
                                   accum_out=st1[:, hh:hh + 1])
                    act.activation(scrA[:], xsb[:, hh, 2:34, :], AF.Square,
                                   accum_out=st1[:, 2 + hh:3 + hh])
                # cc bounce DMAs ride the otherwise-idle gpsimd queue so they
                # are not serialized behind the weight loads on sync's queue.
                gp.dma_start(cc1i[:], st1[:, 0:4])
                gp.collective_compute(
                    "AllReduce", OP.add, replica_groups=[list(range(N_CORES))],
                    ins=[cc1i.opt()], outs=[cc1o.opt()])
                gp.dma_start(st1[:, 4:8], cc1o[:])

            def edge_mask(buf_row_ap, mcol):
                ve.tensor_scalar(buf_row_ap, buf_row_ap,
                                 bv[:, EMSK + mcol:EMSK + mcol + 1], None,
                                 op0=OP.mult)

            # ---- first convs: h = relu(conv(segmap)+b), K=105 (3 sy packed) ----
            def hconv(hbuf, cidx, jof, nrows, mrows):
                for (r0, nr) in _row_tiles(nrows):
                    npx = nr * 128
                    ps = ph.tile([128, 512], F32, tag="ph", name=f"ps_h{cidx}_{r0}")
                    for sx in range(3):
                        te.matmul(ps[:, :npx], w1[:, sx, cidx, :],
                                  seg[:, jof + r0: jof + r0 + nr, sx:sx + 128],
                                  start=(sx == 0), stop=(sx == 2))
                    act.activation(hbuf[:, r0:r0 + nr, 1:129], ps[:, :npx],
                                   AF.Relu, bias=bv[:, B1C + cidx:B1C + cidx + 1])
                edge_mask(hbuf[:, mrows[0], :], 0)
                edge_mask(hbuf[:, mrows[1], :], 1)

            # ---- mean-removal: c = mean(h); dq = h - c (fp8, pads -> -c);
            # bias_g = (Wsum^T c)_g + b2_g via ng tiny matmuls ----
            # scratch columns for the per-h sums (consumed by the DVE mul
            # before bn_coeffs overwrites them; DVE is in-order)
            MR_STC = {0: (st1, 8), 1: (st2, 8), 2: (st1, 10), 3: (st2, 10)}

            def mean_stats(hbuf, dqbuf, ccol, nrows):
                # ScalarE/DVE only: emitted right after the h conv so it runs
                # while the PE continues with the next conv.
                scr = cwtp.tile([128, 36, 130], F16, tag="cwt", name=f"scrc{ccol}")
                stb, sc0 = MR_STC[ccol]
                act.activation(scr[:, :nrows, :], hbuf[:, :nrows, :], AF.Identity,
                               accum_out=stb[:, sc0:sc0 + 1])
                # c is materialized in fp16 (for the fp16 bias matvec) and
                # upcast to an exact fp32 twin for the DVE subtract, so both
                # consumers see bit-identical c and the correction is exact.
                ve.tensor_scalar_mul(cv[:, ccol:ccol + 1], stb[:, sc0:sc0 + 1],
                                     1.0 / (nrows * 130.0))
                act.activation(cv32[:, ccol:ccol + 1], cv[:, ccol:ccol + 1],
                               AF.Identity)
                ve.tensor_scalar(dqbuf[:, :nrows, :], hbuf[:, :nrows, :],
                                 cv32[:, ccol:ccol + 1], None, op0=OP.subtract)

            def mean_bias(ccol, wstile, wsj, btbuf, b2c0, ng):
                # PE matvec: emitted late enough that c is already computed,
                # so the in-order PE never blocks on the stats chain.
                # One accumulation group writing disjoint columns: start only
                # on g=0 (start marks the whole 2KB PSUM bank pending-zero;
                # later columns first-touch-zero their own bytes).
                psb = ph.tile([128, ng], F32, tag="ph", name=f"psb{ccol}")
                for g in range(ng):
                    te.matmul(psb[:, g:g + 1], wstile[:, wsj, g, :],
                              cv[:, ccol:ccol + 1], start=(g == 0),
                              stop=(g == ng - 1), skip_group_check=True)
                ve.tensor_add(btbuf[:], psb[:], bv[:, b2c0:b2c0 + ng])

            # ---- se conv: sw = sigmoid(conv3x3+b), fp8 DoubleRow on the
            # mean-removed dq (4 pairs + 1 single, scale-32 weights) ----
            def seconv(swbuf, dqsrc, sesb, btse, nrows):
                for (r0, nr) in _row_tiles(nrows):
                    npx = nr * 128
                    for hh in range(2):
                        ps = pcw.tile([128, 512], F32, tag="pcw",
                                      name=f"ps_se{nrows}_{r0}_{hh}")
                        for pi, (pa, pb) in enumerate(GW2_PAIRS):
                            sy0, sx0 = divmod(pa, 3)
                            sy1, sx1 = divmod(pb, 3)
                            a = dqsrc[:, r0 + sy0: r0 + sy0 + nr, sx0:sx0 + 128]
                            delta = (sy1 - sy0) * 130 + (sx1 - sx0)
                            rhs = APc(a.tensor, a.offset,
                                      [list(a.ap[0]), [delta, 2], [130, nr],
                                       [1, 128]])
                            te.matmul(ps[:, :npx], sesb[:, pa:pa + 2, hh, :], rhs,
                                      start=(pi == 0), stop=False, perf_mode=DR,
                                      skip_group_check=True)
                        te.matmul(ps[:, :npx], sesb[:, 8, hh, :],
                                  dqsrc[:, r0 + 2: r0 + 2 + nr, 2:2 + 128],
                                  start=False, stop=True, skip_group_check=True)
                        act.activation(swbuf[:, hh, r0:r0 + nr, :], ps[:, :npx],
                                       AF.Sigmoid, scale=1.0 / WSC,
                                       bias=btse[:, hh:hh + 1])

            h2 = static.tile([128, 34, 130], F16, tag="hA", name="h2sb")
            dq2 = static.tile([128, 34, 130], F8, tag="dqA", name="dq2sb")
            if level >= 2:
                hconv(h1, 0, 0, 36, (1, 34))
                if level >= 3.1:
                    mean_stats(h1, dq1, 0, 36)
                hconv(h1se, 2, 0, 36, (1, 34))
                if level >= 3.1:
                    mean_stats(h1se, dq1se, 2, 36)
                    # c1 completed while the PE ran h1se's convs
                    mean_bias(0, ws, 0, bt1, GB1, 18)
            if level >= 5:
                # h2 computed early: its slot (h1se's) frees after the sw1
                # conv, this PE work extends the window that hides the BN1
                # AllReduce, and it removes the h2 stall at the B->C boundary.
                zero_pads(h2)
                hconv(h2, 1, 1, 34, (0, 33))
                mean_stats(h2, dq2, 1, 34)
                mean_bias(2, wse, 0, bt1se, SB1, 2)  # c1se ready during h2
            if 3.1 <= level < 5:
                mean_bias(2, wse, 0, bt1se, SB1, 2)
            bn1_stats()
            if level >= 3:
                seconv(sw1, dq1se, se1, bt1se, 34)
            if level >= 5:
                mean_bias(1, ws, 1, bt2, GB2, 18)    # c2 ready during sw1

            # ---- BN coefficient computation (tiny [128,2] ops) ----
            def bn_coeffs(st, gcol, bcol):
                ve.tensor_scalar_mul(st[:, 8:10], st[:, 4:6], 1.0 / CNT)     # mu
                ve.tensor_scalar_mul(st[:, 10:12], st[:, 6:8], 1.0 / CNT)    # q
                ve.tensor_mul(st[:, 12:14], st[:, 8:10], st[:, 8:10])        # mu^2
                ve.tensor_sub(st[:, 12:14], st[:, 10:12], st[:, 12:14])      # var
                ve.tensor_scalar_add(st[:, 12:14], st[:, 12:14], EPS)        # +eps
                act.activation(st[:, 14:16], st[:, 12:14], AF.Sqrt)
                ve.reciprocal(st[:, 16:18], st[:, 14:16])                    # rstd
                ve.tensor_mul(st[:, 18:20], bv[:, gcol:gcol + 2], st[:, 16:18])  # a
                ve.tensor_mul(st[:, 20:22], st[:, 8:10], st[:, 18:20])
                ve.tensor_sub(st[:, 20:22], bv[:, bcol:bcol + 2], st[:, 20:22])  # b
            # NOTE: bn stats sums land in st[:,4:8] post-allreduce; mean_remove
            # uses st1[:,8:9]/st2[:,8:9] as scratch for the h sums BEFORE
            # bn_coeffs overwrites st[:,8:10]; ordering below guarantees the
            # c values are consumed (cast+bias matmuls) before bn_coeffs runs.

            if level >= 3:
                bn_coeffs(st1, BN1G, BN1B)
                # xbn = a1*x + bb1 (interior cols), then edge-row masks.
                # One half on ScalarE, one on DVE to halve the latency.
                act.activation(xbn[:, 0, :, 1:129], xsb[:, 0, :, :],
                               AF.Identity, scale=st1[:, 18:19],
                               bias=st1[:, 20:21])
                ve.tensor_scalar(xbn[:, 1, :, 1:129], xsb[:, 1, :, :],
                                 st1[:, 19:20], st1[:, 21:22],
                                 op0=OP.mult, op1=OP.add)
                edge_mask(xbn[:, 0, 1, :], 0)
                edge_mask(xbn[:, 1, 1, :], 0)
                edge_mask(xbn[:, 0, 34, :], 1)
                edge_mask(xbn[:, 1, 34, :], 1)

            # ---- fused half-block: conv2(gw) -> depthwise -> 1x1 -> gate ----
            LAG = 1

            def gw_matmuls(ps, npx, gwsb, gwf, pairs, f16taps, dqsrc, hsrc,
                           r0, nr, g):
                # DoubleRow fp8 passes (paired taps on the mean-removed dq)
                # then plain fp16 passes (leftover taps on zero-padded h).
                for pi, (pa, pb) in enumerate(pairs):
                    sy0, sx0 = divmod(pa, 3)
                    sy1, sx1 = divmod(pb, 3)
                    a = dqsrc[:, r0 + sy0: r0 + sy0 + nr, sx0:sx0 + 128]
                    delta = (sy1 - sy0) * 130 + (sx1 - sx0)
                    rhs = APc(a.tensor, a.offset,
                              [list(a.ap[0]), [delta, 2], [130, nr], [1, 128]])
                    te.matmul(ps[:, :npx], gwsb[:, pa:pa + 2, g, :], rhs,
                              start=(pi == 0), stop=False, perf_mode=DR,
                              skip_group_check=True)
                for fi, t in enumerate(f16taps):
                    sy, sx = divmod(t, 3)
                    te.matmul(ps[:, :npx], gwf[:, fi, g, :],
                              hsrc[:, r0 + sy: r0 + sy + nr, sx:sx + 128],
                              start=False, stop=(fi == len(f16taps) - 1),
                              skip_group_check=True)

            def halfblock(first, gwsb, gwf, pairs, f16taps, dqsrc, hsrc, xsrc,
                          btbuf, csb, cbc, swsrc, swrows,
                          do_dc=True, do_flush=True, defer_tail=False,
                          on_flush=None):
                tiles = _row_tiles(ROWS_B if first else ROWS_C)
                if not first:
                    # split the final tile so the end-of-kernel pipeline drain
                    # (depthconv + 1x1 + gate + residual) is half as deep
                    (r0l, _) = tiles[-1]
                    tiles = tiles[:-1] + [(r0l, 2), (r0l + 2, 2)]
                pend = []

                def flush_one(item):
                    idx, r0, nr, dps = item
                    npx = nr * 128
                    if not do_flush:
                        return
                    for hh in range(2):
                        up = pu.tile([128, 512], F32, tag="pu",
                                     name=f"up{int(first)}_{idx}_{hh}")
                        te.matmul(up[:, :npx], csb[:, 0, hh, :], dps[0][:, :nr, :],
                                  start=True, stop=False)
                        te.matmul(up[:, :npx], csb[:, 1, hh, :], dps[1][:, :nr, :],
                                  start=False, stop=True)
                        z = workp.tile([128, 512], F16, tag="z", bufs=1,
                                       name=f"z{int(first)}_{idx}_{hh}")
                        ve.scalar_tensor_tensor(
                            z[:, :npx], up[:, :npx], bv[:, cbc + hh:cbc + hh + 1],
                            swsrc[:, hh, r0:r0 + nr, :],
                            op0=OP.add, op1=OP.mult)
                        if first:
                            ve.scalar_tensor_tensor(
                                dxm[:, hh, r0:r0 + nr, 1:129], z[:, :npx], 0.2,
                                z[:, :npx], op0=OP.mult, op1=OP.max)
                        else:
                            d2 = workp.tile([128, 512], F16, tag="d2", bufs=1,
                                            name=f"d2_{idx}_{hh}")
                            ve.scalar_tensor_tensor(
                                d2[:, :npx], z[:, :npx], 0.2, z[:, :npx],
                                op0=OP.mult, op1=OP.max)
                            xrt = workp.tile([128, 512], F32, tag="xr",
                                             name=f"xr_{idx}_{hh}")
                            sync.dma_start(xrt[:, :npx], xr_d.ap()[:, hh, r0:r0 + nr, :])
                            stg = workp.tile([128, 512], F32, tag="st",
                                             name=f"st_{idx}_{hh}")
                            ve.tensor_add(stg[:, :npx], d2[:, :npx], xrt[:, :npx])
                            sync.dma_start(out_d.ap()[:, hh, r0:r0 + nr, :], stg[:, :npx])
                    if on_flush is not None:
                        on_flush(idx, r0, nr)

                for idx, (r0, nr) in enumerate(tiles):
                    npx = nr * 128
                    cwt = cwtp.tile([128, 18, 512], F16, tag="cwt",
                                    name=f"cwt{int(first)}_{idx}")
                    # on the last two tiles of the output phase, alternate the
                    # PSUM drains between ScalarE and DVE: halves the drain
                    # latency on the end-of-kernel critical path
                    split_drain = (not first) and idx >= len(tiles) - 2
                    for g in range(18):
                        ps = pcw.tile([128, 512], F32, tag="pcw",
                                      name=f"pcw{int(first)}_{idx}_{g}")
                        gw_matmuls(ps, npx, gwsb, gwf, pairs, f16taps,
                                   dqsrc, hsrc, r0, nr, g)
                        if split_drain and g % 2 == 1:
                            ve.tensor_scalar(cwt[:, g, :npx], ps[:, :npx],
                                             1.0 / WSC, btbuf[:, g:g + 1],
                                             op0=OP.mult, op1=OP.add)
                        else:
                            act.activation(cwt[:, g, :npx], ps[:, :npx],
                                           AF.Identity, scale=1.0 / WSC,
                                           bias=btbuf[:, g:g + 1])
                    dps = []
                    for hh in range(2 if do_dc else 0):
                        dp = workp.tile([128, 4, 128], F16, tag=f"dp{hh}",
                                        name=f"dp{int(first)}_{idx}_{hh}")
                        tmp = workp.tile([128, 512], F16, tag="tmp", bufs=1,
                                         name=f"tmp{int(first)}_{idx}_{hh}")
                        for t in range(9):
                            ty, tx = divmod(t, 3)
                            src = xsrc[:, hh, r0 + ty: r0 + ty + nr, tx:tx + 128]
                            if t == 0:
                                ve.tensor_mul(dp[:, :nr, :], cwt[:, hh, :npx], src)
                            else:
                                ve.tensor_mul(tmp[:, :npx], cwt[:, t * 2 + hh, :npx], src)
                                ve.tensor_add(dp[:, :nr, :], dp[:, :nr, :], tmp[:, :npx])
                        dps.append(dp)
                    pend.append((idx, r0, nr, dps))
                    if len(pend) > LAG:
                        flush_one(pend.pop(0))
                if defer_tail:
                    return pend, flush_one
                while pend:
                    flush_one(pend.pop(0))
                return None

            if level < 6:
                bstg = static.tile([128, 128], F32, tag="bstg", name="bstg")
                ve.memset(bstg[:], 0.0)
                sync.dma_start(out_d.ap()[:, 0, 0, :], bstg[:])

            # incremental BN2 stats: accumulate partial sums per flushed dxm
            # chunk so the AllReduce can start right after the last flush
            # instead of after four full-height stats passes.
            stp = static.tile([128, 4, 3], F32, tag="stpc", name="stpsb")
            BN2_CHUNKS = {2: (0, 1, 12), 5: (1, 12, 24), 8: (2, 24, 33)}

            def bn2_inc(idx, r0, nr):
                if level < 3.8 or idx not in BN2_CHUNKS:
                    return
                ci, lo, hi = BN2_CHUNKS[idx]
                for hh in range(2):
                    scr = cwtp.tile([128, 12, 130], F16, tag="cwt",
                                    name=f"scrB{ci}_{hh}")
                    nrw = hi - lo
                    act.activation(scr[:, :nrw, :], dxm[:, hh, lo:hi, :],
                                   AF.Identity, accum_out=stp[:, hh, ci:ci + 1])
                    act.activation(scr[:, :nrw, :], dxm[:, hh, lo:hi, :],
                                   AF.Square, accum_out=stp[:, 2 + hh, ci:ci + 1])

            tailB = None
            if level >= 3.2:
                tailB = halfblock(True, gw1, gw1f, GW1_PAIRS, GW1_F16, dq1, h1,
                                  xbn, bt1, c1, C1B, sw1, ROWS_B,
                                  do_dc=(level >= 3.4), do_flush=(level >= 3.6),
                                  defer_tail=(level >= 6),
                                  on_flush=bn2_inc if level >= 3.6 else None)

            if level >= 5:
                # Emit phase-C fmap convs before draining B's tail flushes so
                # the PE has independent work while the DVE finishes phase B.
                gw2 = static.tile([128, 8, 18, 128], F8, tag="gwA", name="gw2sb")
                sync.dma_start(gw2[:], gw2_d.ap())
                gw2f = static.tile([128, 1, 18, 128], F16, tag="gwF", name="gw2fsb")
                sync.dma_start(gw2f[:], gw2f_d.ap())
                se2 = static.tile([128, 9, 2, 128], F8, tag="seC", name="se2sb")
                sync.dma_start(se2[:], se2_d.ap())
                h2se = static.tile([128, 34, 130], F16, tag="hB", name="h2sesb")
                dq2se = static.tile([128, 34, 130], F8, tag="dqS", name="dq2sesb")
                sw2 = static.tile([128, 2, 32, 128], F16, tag="swc", name="sw2sb")
                bn2dx = static.tile([128, 2, 34, 130], F16, tag="wbig2",
                                    name="bn2dxsb")
                zero_pads(h2se)
                zero_pads(bn2dx)
                hconv(h2se, 3, 1, 34, (0, 33))
                mean_stats(h2se, dq2se, 3, 34)
                mean_bias(3, wse, 1, bt2se, SB2, 2)
                seconv(sw2, dq2se, se2, bt2se, 32)

            if tailB is not None:
                pendB, flushB = tailB
                while pendB:
                    flushB(pendB.pop(0))

            if level >= 3.8:
                # reduce the 3 per-chunk partial sums into st2[:, 0:4]
                # (layout stp[128, stat j, chunk]: j = {sum_h0,sum_h1,sq_h0,sq_h1})
                ve.tensor_add(st2[:, 0:4], stp[:, :, 0], stp[:, :, 1])
                ve.tensor_add(st2[:, 0:4], st2[:, 0:4], stp[:, :, 2])
            if level >= 4:
                gp.dma_start(cc2i[:], st2[:, 0:4])
                gp.collective_compute(
                    "AllReduce", OP.add, replica_groups=[list(range(N_CORES))],
                    ins=[cc2i.opt()], outs=[cc2o.opt()])
                gp.dma_start(st2[:, 4:8], cc2o[:])

            if level >= 5:
                bn_coeffs(st2, BN2G, BN2B)
                # one half ScalarE, one DVE: halves the post-AllReduce latency
                act.activation(bn2dx[:, 0, :, 1:129], dxm[:, 0, :, 1:129],
                               AF.Identity, scale=st2[:, 18:19],
                               bias=st2[:, 20:21])
                ve.tensor_scalar(bn2dx[:, 1, :, 1:129], dxm[:, 1, :, 1:129],
                                 st2[:, 19:20], st2[:, 21:22],
                                 op0=OP.mult, op1=OP.add)
                edge_mask(bn2dx[:, 0, 0, :], 0)
                edge_mask(bn2dx[:, 1, 0, :], 0)
                edge_mask(bn2dx[:, 0, 33, :], 1)
                edge_mask(bn2dx[:, 1, 33, :], 1)

            if level >= 6:
                halfblock(False, gw2, gw2f, GW2_PAIRS, GW2_F16, dq2, h2,
                          bn2dx, bt2, c3, C3B, sw2, ROWS_C)

    nc.compile()
    return nc


# ---------------------------------------------------------------------------
# Host-side sharding / layout prep
# ---------------------------------------------------------------------------

def _prep_weights(inp):
    """Shared (shard-independent) weight/bias layout prep."""
    def f16a(a):
        return np.ascontiguousarray(a.astype(F16NP))

    out = {}
    # first convs, K=105 (sy*35+cin), per sx, per conv j in (gw1, gw2, se1, se2)
    w1l = np.zeros((105, 3, 4, 128), np.float32)
    for j, wkey in enumerate(("gw1_w1", "gw2_w1", "se1_w1", "se2_w1")):
        wj = inp[wkey]  # (128, 35, 3, 3)
        for sy in range(3):
            for sx in range(3):
                w1l[sy * 35:(sy + 1) * 35, sx, j, :] = wj[:, :, sy, sx].T
    out["w1l"] = f16a(w1l)

    m = np.arange(128)
    wsum = np.zeros((128, 2, 18, 128), np.float32)
    n8 = {0: 6, 1: 8}  # fp8 tap count per conv (rest go to the fp16 tensor)
    for jj, (key, fkey, src) in enumerate((("gw1l", "gw1f", "gw1_w2"),
                                           ("gw2l", "gw2f", "gw2_w2"))):
        wsrc = inp[src]  # (2304, 128, 3, 3)
        gl = np.zeros((128, 9, 18, 128), np.float32)
        for s9 in range(9):
            sy, sx = divmod(s9, 3)
            for g in range(18):
                t, hh = g // 2, g % 2
                rows = (hh * 128 + m) * 9 + t
                gl[:, s9, g, :] = wsrc[rows, :, sy, sx].T
        k = n8[jj]
        # c-trick bias uses the exact fp32 tap-sum of the fp8 taps only
        wsum[:, jj, :, :] = gl[:, :k].sum(axis=1)
        out[key] = np.ascontiguousarray((gl[:, :k] * WSC).astype(F8NP))
        # fp16 leftover taps pre-scaled by 32 (power of two: exact in fp16)
        # so the uniform 1/32 drain scale applies to the whole PSUM
        out[fkey] = np.ascontiguousarray((gl[:, k:] * WSC).astype(F16NP))
    out["wsuml"] = f16a(wsum)

    wse = np.zeros((128, 2, 2, 128), np.float32)
    for jj, (key, src) in enumerate((("se1l", "se1_w2"), ("se2l", "se2_w2"))):
        wsrc = inp[src]  # (256, 128, 3, 3)
        sl = np.zeros((128, 9, 2, 128), np.float32)
        for s9 in range(9):
            sy, sx = divmod(s9, 3)
            for hh in range(2):
                sl[:, s9, hh, :] = wsrc[hh * 128 + m, :, sy, sx].T
        wse[:, jj, :, :] = sl.sum(axis=1)
        out[key] = np.ascontiguousarray((sl * WSC).astype(F8NP))
    out["wssel"] = f16a(wse)

    for key, src in (("c1l", "conv1_w"), ("c3l", "conv3_w")):
        wsrc = inp[src][:, :, 0, 0]  # (256, 256) [cout, cin]
        cl = np.zeros((128, 2, 2, 128), np.float32)
        for kg in range(2):
            for hh in range(2):
                # cl[k, kg, hh, mo] = w[hh*128+mo, kg*128+k]
                cl[:, kg, hh, :] = wsrc[hh * 128:(hh + 1) * 128,
                                        kg * 128:(kg + 1) * 128].T
        out[key] = f16a(cl)
    return out


def _prep_bvec(inp, top_edge, bot_edge):
    bvec = np.zeros((128, 58), np.float32)
    for j, k in enumerate(("gw1_b1", "gw2_b1", "se1_b1", "se2_b1")):
        bvec[:, j] = inp[k]
    for base, k in ((4, "gw1_b2"), (22, "gw2_b2")):
        b2 = inp[k]
        for g in range(18):
            t, hh = g // 2, g % 2
            bvec[:, base + g] = b2[(hh * 128 + np.arange(128)) * 9 + t]
    for base, k in ((40, "se1_b2"), (42, "se2_b2"), (44, "conv1_b"), (46, "conv3_b"),
                    (48, "bn1_g"), (50, "bn1_b"), (52, "bn2_g"), (54, "bn2_b")):
        v = inp[k]
        bvec[:, base] = v[:128]
        bvec[:, base + 1] = v[128:]
    bvec[:, 56] = 0.0 if top_edge else 1.0
    bvec[:, 57] = 0.0 if bot_edge else 1.0
    return bvec


def _prep_shard(inp, i, wshared):
    n, blk = i // 4, i % 4
    s = RB * blk
    e = s + RB
    x = inp["x"][n]                       # (256,128,128)
    seg_ds = inp["seg"][n][:, ::2, ::2]   # (35,128,128)

    xg = x.reshape(2, 128, H, W)

    x_bf = np.zeros((128, 2, 36, 128), F16NP)
    lo, hi = max(s - 2, 0), min(e + 2, H)
    x_bf[:, :, lo - (s - 2):hi - (s - 2), :] = \
        xg[:, :, lo:hi, :].transpose(1, 0, 2, 3).astype(F16NP)

    x_res = np.ascontiguousarray(
        xg[:, :, s:e, :].transpose(1, 0, 2, 3).astype(np.float32))

    seg3 = np.zeros((105, 36, 130), F16NP)
    seg_f = seg_ds.astype(F16NP)
    for sy in range(3):
        # seg3[sy*35+c, j, 1+x] = segmap[c, s-3+j+sy, x]
        r0 = s - 3 + sy
        lo, hi = max(r0, 0), min(r0 + 36, H)
        if hi > lo:
            seg3[sy * 35:(sy + 1) * 35, lo - r0:hi - r0, 1:129] = seg_f[:, lo:hi, :]

    return {
        "x_bf": x_bf,
        "x_res": x_res,
        "seg3": np.ascontiguousarray(seg3),
        "bvec": _prep_bvec(inp, s == 0, e == H),
        **wshared,
    }


def kernel(**inputs):
    inp = {k: np.asarray(v) for k, v in inputs.items()}

    if "nc" not in _CACHE:
        _CACHE["nc"] = _build()
    nc = _CACHE["nc"]

    wshared = _prep_weights(inp)
    in_maps = [_prep_shard(inp, i, wshared) for i in range(N_CORES)]

    from concourse.bass_utils import run_bass_kernel_spmd
    res = run_bass_kernel_spmd(nc, in_maps, core_ids=list(range(N_CORES)),
                               trace=False)

    out = np.zeros((N, C, H, W), np.float32)
    for i in range(N_CORES):
        n, blk = i // 4, i % 4
        s = RB * blk
        o = res.results[i]["out"]  # (128, 2, 32, 128)
        out[n, :, s:s + RB, :] = o.transpose(1, 0, 2, 3).reshape(C, RB, W)
    return out


# revision 43
# speedup vs baseline: 1.0164x; 1.0164x over previous
"""Trainium2 Bass kernel for nn_DepthsepCCBlock (dense_cnn).

Strategy: 8-way shard over (batch=2) x (H/4 blocks of 32 rows). Each core
computes its 32 output rows end-to-end. The two training-mode BatchNorms
become sync-BN via two tiny (128x4 fp32) AllReduces, both fully overlapped
with TensorEngine work. The dominant 128->2304 3x3 convs run in fp8 e4m3
DoubleRow mode (two taps contracted per PE pass: 9 taps in 5 passes, 1.8x).
Precision is preserved by mean-removal: the conv input is delta = h - c
(c = per-channel mean, pads become -c automatically since h pads are 0) and
the exact-fp32 tap-sum-of-weights times c is folded back in via a per-group
bias computed on device (18 tiny matmuls). This is cell-wise exact for any
c, so only the fp8 quantization of delta and of the weights remains as
error. All other tensors run in fp16 (same PE/DVE speed as bf16, 8x less
noise). The per-pixel dynamic depthwise 3x3 conv runs on the VectorEngine
as 17 tensor-tensor ops per tile, fused with the producing conv. Halo rows
are recomputed from host-sliced zero-padded shards (no halo exchange);
image-boundary BN-bias artifacts are killed with per-core edge-row masks
supplied as data so every core runs one identical NEFF.
"""
import sys
import types
import numpy as np
import ml_dtypes

if "/opt/trn_rl_repo" not in sys.path:
    sys.path.insert(0, "/opt/trn_rl_repo")

F16NP = np.float16
F8NP = ml_dtypes.float8_e4m3

N, C, H, W = 2, 256, 128, 128
SNC, NH = 35, 128
EPS = 1e-5
N_CORES = 8
RB = 32                      # output rows per shard
CNT = float(N * H * W)       # BN reduction count per channel
WSC = 32.0                   # fp8 weight scale

ROWS_B = 34                  # dx_mid rows (s-1 .. e)
ROWS_C = 32                  # output rows (s .. e-1)

# fp8 tap allocation per gw conv: gw1 taps 0-5 via 3 DoubleRow pairs (taps
# 6,7,8 stay fp16), gw2 taps 0-7 via 4 pairs (tap 8 fp16). Chosen from the
# precision sim: (6,8) taps fp8 -> rel err ~1.6e-2 vs 1.84e-2 for (9,9),
# at the cost of one extra PE pass on gw1 only.
GW1_PAIRS = [(0, 1), (2, 3), (4, 5)]
GW1_F16 = [6, 7, 8]
GW2_PAIRS = [(0, 1), (2, 3), (4, 5), (6, 7)]
GW2_F16 = [8]


def _row_tiles(nrows):
    out = []
    r = 0
    while r < nrows:
        nr = min(4, nrows - r)
        out.append((r, nr))
        r += nr
    return out


_CACHE = {}


def _build(level=6):
    from contextlib import ExitStack
    import concourse.tile as tile
    from concourse import bacc, mybir
    from concourse.ap import AP as APc

    F32 = mybir.dt.float32
    F16 = mybir.dt.float16
    F8 = mybir.dt.float8e4
    AF = mybir.ActivationFunctionType
    OP = mybir.AluOpType
    DR = mybir.MatmulPerfMode.DoubleRow

    nc = bacc.Bacc("TRN2", target_bir_lowering=False, debug=False,
                   num_devices=N_CORES)

    x_d = nc.dram_tensor("x_bf", [128, 2, 36, 128], F16, kind="ExternalInput")
    xr_d = nc.dram_tensor("x_res", [128, 2, 32, 128], F32, kind="ExternalInput")
    seg_d = nc.dram_tensor("seg3", [105, 36, 130], F16, kind="ExternalInput")
    w1_d = nc.dram_tensor("w1l", [105, 3, 4, 128], F16, kind="ExternalInput")
    gw1_d = nc.dram_tensor("gw1l", [128, 6, 18, 128], F8, kind="ExternalInput")
    gw2_d = nc.dram_tensor("gw2l", [128, 8, 18, 128], F8, kind="ExternalInput")
    gw1f_d = nc.dram_tensor("gw1f", [128, 3, 18, 128], F16, kind="ExternalInput")
    gw2f_d = nc.dram_tensor("gw2f", [128, 1, 18, 128], F16, kind="ExternalInput")
    ws_d = nc.dram_tensor("wsuml", [128, 2, 18, 128], F16, kind="ExternalInput")
    wse_d = nc.dram_tensor("wssel", [128, 2, 2, 128], F16, kind="ExternalInput")
    se1_d = nc.dram_tensor("se1l", [128, 9, 2, 128], F8, kind="ExternalInput")
    se2_d = nc.dram_tensor("se2l", [128, 9, 2, 128], F8, kind="ExternalInput")
    c1_d = nc.dram_tensor("c1l", [128, 2, 2, 128], F16, kind="ExternalInput")
    c3_d = nc.dram_tensor("c3l", [128, 2, 2, 128], F16, kind="ExternalInput")
    bv_d = nc.dram_tensor("bvec", [128, 58], F32, kind="ExternalInput")
    out_d = nc.dram_tensor("out", [128, 2, 32, 128], F32, kind="ExternalOutput")

    # bvec columns
    B1C = 0          # 4: first-conv biases (h1, h2, h1se, h2se order: gw1,gw2,se1,se2)
    GB1 = 4          # 18: gw1_b2 per (t,half)
    GB2 = 22         # 18: gw2_b2
    SB1 = 40         # 2: se1_b2 ; SB2=42 ; C1B=44 ; C3B=46
    SB2, C1B, C3B = 42, 44, 46
    BN1G, BN1B, BN2G, BN2B = 48, 50, 52, 54
    EMSK = 56        # 2: top,bot edge masks

    with tile.TileContext(nc) as tc:
        with ExitStack() as ctx:
            static = ctx.enter_context(tc.tile_pool(name="static", bufs=1))
            cwtp = ctx.enter_context(tc.tile_pool(name="cwtp", bufs=2))
            workp = ctx.enter_context(tc.tile_pool(name="work", bufs=2))
            dramp = ctx.enter_context(tc.tile_pool(name="dramp", bufs=1, space="DRAM"))
            ph = ctx.enter_context(tc.tile_pool(name="ph", bufs=2, space="PSUM"))
            pcw = ctx.enter_context(tc.tile_pool(name="pcw", bufs=4, space="PSUM"))
            pu = ctx.enter_context(tc.tile_pool(name="pu", bufs=2, space="PSUM"))

            # ---- static SBUF tiles ----
            xsb = static.tile([128, 2, 36, 128], F16, tag="wbig2", name="xsb")
            seg = static.tile([105, 36, 130], F16, tag="segc", name="segsb")
            w1 = static.tile([105, 3, 4, 128], F16, tag="w1c", name="w1sb")
            gw1 = static.tile([128, 6, 18, 128], F8, tag="gwA", name="gw1sb")
            gw1f = static.tile([128, 3, 18, 128], F16, tag="gwF", name="gw1fsb")
            ws = static.tile([128, 2, 18, 128], F16, tag="wsc", name="wssb")
            wse = static.tile([128, 2, 2, 128], F16, tag="wsec", name="wsesb")
            se1 = static.tile([128, 9, 2, 128], F8, tag="seC", name="se1sb")
            dq1se = static.tile([128, 36, 130], F8, tag="dqS", name="dq1sesb")
            bt1se = static.tile([128, 2], F32, tag="btsec", name="bt1sesb")
            bt2se = static.tile([128, 2], F32, tag="btse2c", name="bt2sesb")
            c1 = static.tile([128, 2, 2, 128], F16, tag="c1c", name="c1sb")
            c3 = static.tile([128, 2, 2, 128], F16, tag="c3c", name="c3sb")
            bv = static.tile([128, 58], F32, tag="bvc", name="bvsb")
            h1 = static.tile([128, 36, 130], F16, tag="hB", name="h1sb")
            h1se = static.tile([128, 36, 130], F16, tag="hA", name="h1sesb")
            dq1 = static.tile([128, 36, 130], F8, tag="dqA", name="dq1sb")
            xbn = static.tile([128, 2, 36, 130], F16, tag="xbnc", name="xbnsb")
            sw1 = static.tile([128, 2, 34, 128], F16, tag="swc", name="sw1sb")
            dxm = static.tile([128, 2, 34, 130], F16, tag="dxmc", name="dxmsb")
            st1 = static.tile([128, 22], F32, tag="st1c", name="st1sb")
            st2 = static.tile([128, 22], F32, tag="st2c", name="st2sb")
            cv = static.tile([128, 4], F16, tag="cvc", name="cvsb")
            cv32 = static.tile([128, 4], F32, tag="cv32c", name="cv32sb")
            bt1 = static.tile([128, 18], F32, tag="bt1c", name="bt1sb")
            bt2 = static.tile([128, 18], F32, tag="bt2c", name="bt2sb")

            cc1i = dramp.tile([128, 4], F32, name="cc1i")
            cc1o = dramp.tile([128, 4], F32, addr_space="Shared", name="cc1o")
            cc2i = dramp.tile([128, 4], F32, name="cc2i")
            cc2o = dramp.tile([128, 4], F32, addr_space="Shared", name="cc2o")

            sync, ve, act, gp, te = nc.sync, nc.vector, nc.scalar, nc.gpsimd, nc.tensor

            # ---- input DMAs. One queue, critical-first: the first conv needs
            # only seg/w1/bv, and a single in-order queue guarantees the big
            # loads don't steal HBM bandwidth from them (a multi-queue spread
            # measured 23us slower to first matmul).
            sync.dma_start(seg[:], seg_d.ap())
            sync.dma_start(w1[:], w1_d.ap())
            sync.dma_start(bv[:], bv_d.ap())
            sync.dma_start(xsb[:], x_d.ap())
            sync.dma_start(se1[:], se1_d.ap())
            sync.dma_start(ws[:], ws_d.ap())
            sync.dma_start(wse[:], wse_d.ap())
            sync.dma_start(gw1[:], gw1_d.ap())
            sync.dma_start(gw1f[:], gw1f_d.ap())
            sync.dma_start(c1[:], c1_d.ap())
            sync.dma_start(c3[:], c3_d.ap())

            # ---- zero pad cells. Every row and all interior columns of these
            # buffers get written by drains/affines before any read, so only
            # the two pad columns need zeroing.
            def zero_pads(buf):
                if len(buf.shape) == 4:
                    ve.memset(buf[:, :, :, 0], 0.0)
                    ve.memset(buf[:, :, :, 129], 0.0)
                else:
                    ve.memset(buf[:, :, 0], 0.0)
                    ve.memset(buf[:, :, 129], 0.0)

            for buf in (h1, h1se, xbn, dxm):
                zero_pads(buf)

            def bn1_stats():
                # Emitted after the h convs: ScalarE is the drain engine for
                # the first-conv PSUMs, and 15us of stats passes up front
                # stalls the PE behind the 2-bank ph pool.
                scrA = cwtp.tile([128, 32, 128], F16, tag="cwt", name="scrA")
                for hh in range(2):
                    act.activation(scrA[:], xsb[:, hh, 2:34, :], AF.Identity,
                                   accum_out=st1[:, hh:hh + 1])
                    act.activation(scrA[:], xsb[:, hh, 2:34, :], AF.Square,
                                   accum_out=st1[:, 2 + hh:3 + hh])
                # cc bounce DMAs ride the otherwise-idle gpsimd queue so they
                # are not serialized behind the weight loads on sync's queue.
                gp.dma_start(cc1i[:], st1[:, 0:4])
                gp.collective_compute(
                    "AllReduce", OP.add, replica_groups=[list(range(N_CORES))],
                    ins=[cc1i.opt()], outs=[cc1o.opt()])
                gp.dma_start(st1[:, 4:8], cc1o[:])

            def edge_mask(buf_row_ap, mcol):
                ve.tensor_scalar(buf_row_ap, buf_row_ap,
                                 bv[:, EMSK + mcol:EMSK + mcol + 1], None,
                                 op0=OP.mult)

            # ---- first convs: h = relu(conv(segmap)+b), K=105 (3 sy packed) ----
            def hconv(hbuf, cidx, jof, nrows, mrows):
                for (r0, nr) in _row_tiles(nrows):
                    npx = nr * 128
                    ps = ph.tile([128, 512], F32, tag="ph", name=f"ps_h{cidx}_{r0}")
                    for sx in range(3):
                        te.matmul(ps[:, :npx], w1[:, sx, cidx, :],
                                  seg[:, jof + r0: jof + r0 + nr, sx:sx + 128],
                                  start=(sx == 0), stop=(sx == 2))
                    act.activation(hbuf[:, r0:r0 + nr, 1:129], ps[:, :npx],
                                   AF.Relu, bias=bv[:, B1C + cidx:B1C + cidx + 1])
                edge_mask(hbuf[:, mrows[0], :], 0)
                edge_mask(hbuf[:, mrows[1], :], 1)

            # ---- mean-removal: c = mean(h); dq = h - c (fp8, pads -> -c);
            # bias_g = (Wsum^T c)_g + b2_g via ng tiny matmuls ----
            # scratch columns for the per-h sums (consumed by the DVE mul
            # before bn_coeffs overwrites them; DVE is in-order)
            MR_STC = {0: (st1, 8), 1: (st2, 8), 2: (st1, 10), 3: (st2, 10)}

            def mean_stats(hbuf, dqbuf, ccol, nrows):
                # ScalarE/DVE only: emitted right after the h conv so it runs
                # while the PE continues with the next conv.
                scr = cwtp.tile([128, 36, 130], F16, tag="cwt", name=f"scrc{ccol}")
                stb, sc0 = MR_STC[ccol]
                act.activation(scr[:, :nrows, :], hbuf[:, :nrows, :], AF.Identity,
                               accum_out=stb[:, sc0:sc0 + 1])
                # c is materialized in fp16 (for the fp16 bias matvec) and
                # upcast to an exact fp32 twin for the DVE subtract, so both
                # consumers see bit-identical c and the correction is exact.
                ve.tensor_scalar_mul(cv[:, ccol:ccol + 1], stb[:, sc0:sc0 + 1],
                                     1.0 / (nrows * 130.0))
                act.activation(cv32[:, ccol:ccol + 1], cv[:, ccol:ccol + 1],
                               AF.Identity)
                ve.tensor_scalar(dqbuf[:, :nrows, :], hbuf[:, :nrows, :],
                                 cv32[:, ccol:ccol + 1], None, op0=OP.subtract)

            def mean_bias(ccol, wstile, wsj, btbuf, b2c0, ng):
                # PE matvec: emitted late enough that c is already computed,
                # so the in-order PE never blocks on the stats chain.
                # One accumulation group writing disjoint columns: start only
                # on g=0 (start marks the whole 2KB PSUM bank pending-zero;
                # later columns first-touch-zero their own bytes).
                psb = ph.tile([128, ng], F32, tag="ph", name=f"psb{ccol}")
                for g in range(ng):
                    te.matmul(psb[:, g:g + 1], wstile[:, wsj, g, :],
                              cv[:, ccol:ccol + 1], start=(g == 0),
                              stop=(g == ng - 1), skip_group_check=True)
                ve.tensor_add(btbuf[:], psb[:], bv[:, b2c0:b2c0 + ng])

            # ---- se conv: sw = sigmoid(conv3x3+b), fp8 DoubleRow on the
            # mean-removed dq (4 pairs + 1 single, scale-32 weights) ----
            def seconv(swbuf, dqsrc, sesb, btse, nrows):
                for (r0, nr) in _row_tiles(nrows):
                    npx = nr * 128
                    for hh in range(2):
                        ps = pcw.tile([128, 512], F32, tag="pcw",
                                      name=f"ps_se{nrows}_{r0}_{hh}")
                        for pi, (pa, pb) in enumerate(GW2_PAIRS):
                            sy0, sx0 = divmod(pa, 3)
                            sy1, sx1 = divmod(pb, 3)
                            a = dqsrc[:, r0 + sy0: r0 + sy0 + nr, sx0:sx0 + 128]
                            delta = (sy1 - sy0) * 130 + (sx1 - sx0)
                            rhs = APc(a.tensor, a.offset,
                                      [list(a.ap[0]), [delta, 2], [130, nr],
                                       [1, 128]])
                            te.matmul(ps[:, :npx], sesb[:, pa:pa + 2, hh, :], rhs,
                                      start=(pi == 0), stop=False, perf_mode=DR,
                                      skip_group_check=True)
                        te.matmul(ps[:, :npx], sesb[:, 8, hh, :],
                                  dqsrc[:, r0 + 2: r0 + 2 + nr, 2:2 + 128],
                                  start=False, stop=True, skip_group_check=True)
                        act.activation(swbuf[:, hh, r0:r0 + nr, :], ps[:, :npx],
                                       AF.Sigmoid, scale=1.0 / WSC,
                                       bias=btse[:, hh:hh + 1])

            h2 = static.tile([128, 34, 130], F16, tag="hA", name="h2sb")
            dq2 = static.tile([128, 34, 130], F8, tag="dqA", name="dq2sb")
            if level >= 2:
                hconv(h1, 0, 0, 36, (1, 34))
                if level >= 3.1:
                    mean_stats(h1, dq1, 0, 36)
                hconv(h1se, 2, 0, 36, (1, 34))
                if level >= 3.1:
                    mean_stats(h1se, dq1se, 2, 36)
                    # c1 completed while the PE ran h1se's convs
                    mean_bias(0, ws, 0, bt1, GB1, 18)
            if level >= 5:
                # h2 computed early: its slot (h1se's) frees after the sw1
                # conv, this PE work extends the window that hides the BN1
                # AllReduce, and it removes the h2 stall at the B->C boundary.
                zero_pads(h2)
                hconv(h2, 1, 1, 34, (0, 33))
                mean_stats(h2, dq2, 1, 34)
                mean_bias(2, wse, 0, bt1se, SB1, 2)  # c1se ready during h2
            if 3.1 <= level < 5:
                mean_bias(2, wse, 0, bt1se, SB1, 2)
            bn1_stats()
            if level >= 3:
                seconv(sw1, dq1se, se1, bt1se, 34)
            if level >= 5:
                mean_bias(1, ws, 1, bt2, GB2, 18)    # c2 ready during sw1

            # ---- BN coefficient computation (tiny [128,2] ops) ----
            def bn_coeffs(st, gcol, bcol):
                ve.tensor_scalar_mul(st[:, 8:10], st[:, 4:6], 1.0 / CNT)     # mu
                ve.tensor_scalar_mul(st[:, 10:12], st[:, 6:8], 1.0 / CNT)    # q
                ve.tensor_mul(st[:, 12:14], st[:, 8:10], st[:, 8:10])        # mu^2
                ve.tensor_sub(st[:, 12:14], st[:, 10:12], st[:, 12:14])      # var
                ve.tensor_scalar_add(st[:, 12:14], st[:, 12:14], EPS)        # +eps
                act.activation(st[:, 14:16], st[:, 12:14], AF.Sqrt)
                ve.reciprocal(st[:, 16:18], st[:, 14:16])                    # rstd
                ve.tensor_mul(st[:, 18:20], bv[:, gcol:gcol + 2], st[:, 16:18])  # a
                ve.tensor_mul(st[:, 20:22], st[:, 8:10], st[:, 18:20])
                ve.tensor_sub(st[:, 20:22], bv[:, bcol:bcol + 2], st[:, 20:22])  # b
            # NOTE: bn stats sums land in st[:,4:8] post-allreduce; mean_remove
            # uses st1[:,8:9]/st2[:,8:9] as scratch for the h sums BEFORE
            # bn_coeffs overwrites st[:,8:10]; ordering below guarantees the
            # c values are consumed (cast+bias matmuls) before bn_coeffs runs.

            if level >= 3:
                bn_coeffs(st1, BN1G, BN1B)
                # xbn = a1*x + bb1 (interior cols), then edge-row masks.
                # One half on ScalarE, one on DVE to halve the latency.
                act.activation(xbn[:, 0, :, 1:129], xsb[:, 0, :, :],
                               AF.Identity, scale=st1[:, 18:19],
                               bias=st1[:, 20:21])
                ve.tensor_scalar(xbn[:, 1, :, 1:129], xsb[:, 1, :, :],
                                 st1[:, 19:20], st1[:, 21:22],
                                 op0=OP.mult, op1=OP.add)
                edge_mask(xbn[:, 0, 1, :], 0)
                edge_mask(xbn[:, 1, 1, :], 0)
                edge_mask(xbn[:, 0, 34, :], 1)
                edge_mask(xbn[:, 1, 34, :], 1)

            # ---- fused half-block: conv2(gw) -> depthwise -> 1x1 -> gate ----
            LAG = 1

            def gw_matmuls(ps, npx, gwsb, gwf, pairs, f16taps, dqsrc, hsrc,
                           r0, nr, g):
                # DoubleRow fp8 passes (paired taps on the mean-removed dq)
                # then plain fp16 passes (leftover taps on zero-padded h).
                for pi, (pa, pb) in enumerate(pairs):
                    sy0, sx0 = divmod(pa, 3)
                    sy1, sx1 = divmod(pb, 3)
                    a = dqsrc[:, r0 + sy0: r0 + sy0 + nr, sx0:sx0 + 128]
                    delta = (sy1 - sy0) * 130 + (sx1 - sx0)
                    rhs = APc(a.tensor, a.offset,
                              [list(a.ap[0]), [delta, 2], [130, nr], [1, 128]])
                    te.matmul(ps[:, :npx], gwsb[:, pa:pa + 2, g, :], rhs,
                              start=(pi == 0), stop=False, perf_mode=DR,
                              skip_group_check=True)
                for fi, t in enumerate(f16taps):
                    sy, sx = divmod(t, 3)
                    te.matmul(ps[:, :npx], gwf[:, fi, g, :],
                              hsrc[:, r0 + sy: r0 + sy + nr, sx:sx + 128],
                              start=False, stop=(fi == len(f16taps) - 1),
                              skip_group_check=True)

            def halfblock(first, gwsb, gwf, pairs, f16taps, dqsrc, hsrc, xsrc,
                          btbuf, csb, cbc, swsrc, swrows,
                          do_dc=True, do_flush=True, defer_tail=False,
                          on_flush=None, after_tile0=None):
                tiles = _row_tiles(ROWS_B if first else ROWS_C)
                if not first:
                    # split the final tile so the end-of-kernel pipeline drain
                    # (depthconv + 1x1 + gate + residual) is half as deep
                    (r0l, _) = tiles[-1]
                    tiles = tiles[:-1] + [(r0l, 2), (r0l + 2, 2)]
                pend = []

                def flush_one(item):
                    idx, r0, nr, dps = item
                    npx = nr * 128
                    if not do_flush:
                        return
                    for hh in range(2):
                        up = pu.tile([128, 512], F32, tag="pu",
                                     name=f"up{int(first)}_{idx}_{hh}")
                        te.matmul(up[:, :npx], csb[:, 0, hh, :], dps[0][:, :nr, :],
                                  start=True, stop=False)
                        te.matmul(up[:, :npx], csb[:, 1, hh, :], dps[1][:, :nr, :],
                                  start=False, stop=True)
                        z = workp.tile([128, 512], F16, tag="z", bufs=1,
                                       name=f"z{int(first)}_{idx}_{hh}")
                        ve.scalar_tensor_tensor(
                            z[:, :npx], up[:, :npx], bv[:, cbc + hh:cbc + hh + 1],
                            swsrc[:, hh, r0:r0 + nr, :],
                            op0=OP.add, op1=OP.mult)
                        if first:
                            ve.scalar_tensor_tensor(
                                dxm[:, hh, r0:r0 + nr, 1:129], z[:, :npx], 0.2,
                                z[:, :npx], op0=OP.mult, op1=OP.max)
                        else:
                            d2 = workp.tile([128, 512], F16, tag="d2", bufs=1,
                                            name=f"d2_{idx}_{hh}")
                            ve.scalar_tensor_tensor(
                                d2[:, :npx], z[:, :npx], 0.2, z[:, :npx],
                                op0=OP.mult, op1=OP.max)
                            xrt = workp.tile([128, 512], F32, tag="xr",
                                             name=f"xr_{idx}_{hh}")
                            sync.dma_start(xrt[:, :npx], xr_d.ap()[:, hh, r0:r0 + nr, :])
                            stg = workp.tile([128, 512], F32, tag="st",
                                             name=f"st_{idx}_{hh}")
                            ve.tensor_add(stg[:, :npx], d2[:, :npx], xrt[:, :npx])
                            sync.dma_start(out_d.ap()[:, hh, r0:r0 + nr, :], stg[:, :npx])
                    if on_flush is not None:
                        on_flush(idx, r0, nr)

                for idx, (r0, nr) in enumerate(tiles):
                    npx = nr * 128
                    cwt = cwtp.tile([128, 18, 512], F16, tag="cwt",
                                    name=f"cwt{int(first)}_{idx}")
                    for g in range(18):
                        ps = pcw.tile([128, 512], F32, tag="pcw",
                                      name=f"pcw{int(first)}_{idx}_{g}")
                        gw_matmuls(ps, npx, gwsb, gwf, pairs, f16taps,
                                   dqsrc, hsrc, r0, nr, g)
                        act.activation(cwt[:, g, :npx], ps[:, :npx],
                                       AF.Identity, scale=1.0 / WSC,
                                       bias=btbuf[:, g:g + 1])
                    if idx == 0 and after_tile0 is not None:
                        # AR-independent PE work emitted before the first
                        # flush: covers the BN2-AllReduce wait at the B->C
                        # boundary (the in-order PE would otherwise stall at
                        # the first 1x1, which depends on bn2dx)
                        after_tile0()
                    dps = []
                    for hh in range(2 if do_dc else 0):
                        dp = workp.tile([128, 4, 128], F16, tag=f"dp{hh}",
                                        name=f"dp{int(first)}_{idx}_{hh}")
                        tmp = workp.tile([128, 512], F16, tag="tmp", bufs=1,
                                         name=f"tmp{int(first)}_{idx}_{hh}")
                        for t in range(9):
                            ty, tx = divmod(t, 3)
                            src = xsrc[:, hh, r0 + ty: r0 + ty + nr, tx:tx + 128]
                            if t == 0:
                                ve.tensor_mul(dp[:, :nr, :], cwt[:, hh, :npx], src)
                            else:
                                ve.tensor_mul(tmp[:, :npx], cwt[:, t * 2 + hh, :npx], src)
                                ve.tensor_add(dp[:, :nr, :], dp[:, :nr, :], tmp[:, :npx])
                        dps.append(dp)
                    pend.append((idx, r0, nr, dps))
                    if len(pend) > LAG:
                        flush_one(pend.pop(0))
                if defer_tail:
                    return pend, flush_one
                while pend:
                    flush_one(pend.pop(0))
                return None

            if level < 6:
                bstg = static.tile([128, 128], F32, tag="bstg", name="bstg")
                ve.memset(bstg[:], 0.0)
                sync.dma_start(out_d.ap()[:, 0, 0, :], bstg[:])

            # incremental BN2 stats: accumulate partial sums per flushed dxm
            # chunk so the AllReduce can start right after the last flush
            # instead of after four full-height stats passes.
            stp = static.tile([128, 4, 3], F32, tag="stpc", name="stpsb")
            BN2_CHUNKS = {2: (0, 1, 12), 5: (1, 12, 24), 8: (2, 24, 33)}

            def bn2_inc(idx, r0, nr):
                if level < 3.8 or idx not in BN2_CHUNKS:
                    return
                ci, lo, hi = BN2_CHUNKS[idx]
                for hh in range(2):
                    scr = cwtp.tile([128, 12, 130], F16, tag="cwt",
                                    name=f"scrB{ci}_{hh}")
                    nrw = hi - lo
                    act.activation(scr[:, :nrw, :], dxm[:, hh, lo:hi, :],
                                   AF.Identity, accum_out=stp[:, hh, ci:ci + 1])
                    act.activation(scr[:, :nrw, :], dxm[:, hh, lo:hi, :],
                                   AF.Square, accum_out=stp[:, 2 + hh, ci:ci + 1])

            tailB = None
            if level >= 3.2:
                tailB = halfblock(True, gw1, gw1f, GW1_PAIRS, GW1_F16, dq1, h1,
                                  xbn, bt1, c1, C1B, sw1, ROWS_B,
                                  do_dc=(level >= 3.4), do_flush=(level >= 3.6),
                                  defer_tail=(level >= 6),
                                  on_flush=bn2_inc if level >= 3.6 else None)

            if level >= 5:
                # Emit phase-C fmap convs before draining B's tail flushes so
                # the PE has independent work while the DVE finishes phase B.
                gw2 = static.tile([128, 8, 18, 128], F8, tag="gwA", name="gw2sb")
                sync.dma_start(gw2[:], gw2_d.ap())
                gw2f = static.tile([128, 1, 18, 128], F16, tag="gwF", name="gw2fsb")
                sync.dma_start(gw2f[:], gw2f_d.ap())
                se2 = static.tile([128, 9, 2, 128], F8, tag="seC", name="se2sb")
                sync.dma_start(se2[:], se2_d.ap())
                h2se = static.tile([128, 34, 130], F16, tag="hB", name="h2sesb")
                dq2se = static.tile([128, 34, 130], F8, tag="dqS", name="dq2sesb")
                sw2 = static.tile([128, 2, 32, 128], F16, tag="swc", name="sw2sb")
                bn2dx = static.tile([128, 2, 34, 130], F16, tag="wbig2",
                                    name="bn2dxsb")
                zero_pads(h2se)
                zero_pads(bn2dx)
                hconv(h2se, 3, 1, 34, (0, 33))
                mean_stats(h2se, dq2se, 3, 34)
                mean_bias(3, wse, 1, bt2se, SB2, 2)

            if tailB is not None:
                pendB, flushB = tailB
                while pendB:
                    flushB(pendB.pop(0))

            if level >= 3.8:
                # reduce the 3 per-chunk partial sums into st2[:, 0:4]
                # (layout stp[128, stat j, chunk]: j = {sum_h0,sum_h1,sq_h0,sq_h1})
                ve.tensor_add(st2[:, 0:4], stp[:, :, 0], stp[:, :, 1])
                ve.tensor_add(st2[:, 0:4], st2[:, 0:4], stp[:, :, 2])
            if level >= 4:
                gp.dma_start(cc2i[:], st2[:, 0:4])
                gp.collective_compute(
                    "AllReduce", OP.add, replica_groups=[list(range(N_CORES))],
                    ins=[cc2i.opt()], outs=[cc2o.opt()])
                gp.dma_start(st2[:, 4:8], cc2o[:])

            if level >= 5:
                bn_coeffs(st2, BN2G, BN2B)
                # one half ScalarE, one DVE: halves the post-AllReduce latency
                act.activation(bn2dx[:, 0, :, 1:129], dxm[:, 0, :, 1:129],
                               AF.Identity, scale=st2[:, 18:19],
                               bias=st2[:, 20:21])
                ve.tensor_scalar(bn2dx[:, 1, :, 1:129], dxm[:, 1, :, 1:129],
                                 st2[:, 19:20], st2[:, 21:22],
                                 op0=OP.mult, op1=OP.add)
                edge_mask(bn2dx[:, 0, 0, :], 0)
                edge_mask(bn2dx[:, 1, 0, :], 0)
                edge_mask(bn2dx[:, 0, 33, :], 1)
                edge_mask(bn2dx[:, 1, 33, :], 1)

            if level >= 6:
                halfblock(False, gw2, gw2f, GW2_PAIRS, GW2_F16, dq2, h2,
                          bn2dx, bt2, c3, C3B, sw2, ROWS_C,
                          after_tile0=lambda: seconv(sw2, dq2se, se2, bt2se, 32))

    nc.compile()
    return nc


# ---------------------------------------------------------------------------
# Host-side sharding / layout prep
# ---------------------------------------------------------------------------

def _prep_weights(inp):
    """Shared (shard-independent) weight/bias layout prep."""
    def f16a(a):
        return np.ascontiguousarray(a.astype(F16NP))

    out = {}
    # first convs, K=105 (sy*35+cin), per sx, per conv j in (gw1, gw2, se1, se2)
    w1l = np.zeros((105, 3, 4, 128), np.float32)
    for j, wkey in enumerate(("gw1_w1", "gw2_w1", "se1_w1", "se2_w1")):
        wj = inp[wkey]  # (128, 35, 3, 3)
        for sy in range(3):
            for sx in range(3):
                w1l[sy * 35:(sy + 1) * 35, sx, j, :] = wj[:, :, sy, sx].T
    out["w1l"] = f16a(w1l)

    m = np.arange(128)
    wsum = np.zeros((128, 2, 18, 128), np.float32)
    n8 = {0: 6, 1: 8}  # fp8 tap count per conv (rest go to the fp16 tensor)
    for jj, (key, fkey, src) in enumerate((("gw1l", "gw1f", "gw1_w2"),
                                           ("gw2l", "gw2f", "gw2_w2"))):
        wsrc = inp[src]  # (2304, 128, 3, 3)
        gl = np.zeros((128, 9, 18, 128), np.float32)
        for s9 in range(9):
            sy, sx = divmod(s9, 3)
            for g in range(18):
                t, hh = g // 2, g % 2
                rows = (hh * 128 + m) * 9 + t
                gl[:, s9, g, :] = wsrc[rows, :, sy, sx].T
        k = n8[jj]
        # c-trick bias uses the exact fp32 tap-sum of the fp8 taps only
        wsum[:, jj, :, :] = gl[:, :k].sum(axis=1)
        out[key] = np.ascontiguousarray((gl[:, :k] * WSC).astype(F8NP))
        # fp16 leftover taps pre-scaled by 32 (power of two: exact in fp16)
        # so the uniform 1/32 drain scale applies to the whole PSUM
        out[fkey] = np.ascontiguousarray((gl[:, k:] * WSC).astype(F16NP))
    out["wsuml"] = f16a(wsum)

    wse = np.zeros((128, 2, 2, 128), np.float32)
    for jj, (key, src) in enumerate((("se1l", "se1_w2"), ("se2l", "se2_w2"))):
        wsrc = inp[src]  # (256, 128, 3, 3)
        sl = np.zeros((128, 9, 2, 128), np.float32)
        for s9 in range(9):
            sy, sx = divmod(s9, 3)
            for hh in range(2):
                sl[:, s9, hh, :] = wsrc[hh * 128 + m, :, sy, sx].T
        wse[:, jj, :, :] = sl.sum(axis=1)
        out[key] = np.ascontiguousarray((sl * WSC).astype(F8NP))
    out["wssel"] = f16a(wse)

    for key, src in (("c1l", "conv1_w"), ("c3l", "conv3_w")):
        wsrc = inp[src][:, :, 0, 0]  # (256, 256) [cout, cin]
        cl = np.zeros((128, 2, 2, 128), np.float32)
        for kg in range(2):
            for hh in range(2):
                # cl[k, kg, hh, mo] = w[hh*128+mo, kg*128+k]
                cl[:, kg, hh, :] = wsrc[hh * 128:(hh + 1) * 128,
                                        kg * 128:(kg + 1) * 128].T
        out[key] = f16a(cl)
    return out


def _prep_bvec(inp, top_edge, bot_edge):
    bvec = np.zeros((128, 58), np.float32)
    for j, k in enumerate(("gw1_b1", "gw2_b1", "se1_b1", "se2_b1")):
        bvec[:, j] = inp[k]
    for base, k in ((4, "gw1_b2"), (22, "gw2_b2")):
        b2 = inp[k]
        for g in range(18):
            t, hh = g // 2, g % 2
            bvec[:, base + g] = b2[(hh * 128 + np.arange(128)) * 9 + t]
    for base, k in ((40, "se1_b2"), (42, "se2_b2"), (44, "conv1_b"), (46, "conv3_b"),
                    (48, "bn1_g"), (50, "bn1_b"), (52, "bn2_g"), (54, "bn2_b")):
        v = inp[k]
        bvec[:, base] = v[:128]
        bvec[:, base + 1] = v[128:]
    bvec[:, 56] = 0.0 if top_edge else 1.0
    bvec[:, 57] = 0.0 if bot_edge else 1.0
    return bvec


def _prep_shard(inp, i, wshared):
    n, blk = i // 4, i % 4
    s = RB * blk
    e = s + RB
    x = inp["x"][n]                       # (256,128,128)
    seg_ds = inp["seg"][n][:, ::2, ::2]   # (35,128,128)

    xg = x.reshape(2, 128, H, W)

    x_bf = np.zeros((128, 2, 36, 128), F16NP)
    lo, hi = max(s - 2, 0), min(e + 2, H)
    x_bf[:, :, lo - (s - 2):hi - (s - 2), :] = \
        xg[:, :, lo:hi, :].transpose(1, 0, 2, 3).astype(F16NP)

    x_res = np.ascontiguousarray(
        xg[:, :, s:e, :].transpose(1, 0, 2, 3).astype(np.float32))

    seg3 = np.zeros((105, 36, 130), F16NP)
    seg_f = seg_ds.astype(F16NP)
    for sy in range(3):
        # seg3[sy*35+c, j, 1+x] = segmap[c, s-3+j+sy, x]
        r0 = s - 3 + sy
        lo, hi = max(r0, 0), min(r0 + 36, H)
        if hi > lo:
            seg3[sy * 35:(sy + 1) * 35, lo - r0:hi - r0, 1:129] = seg_f[:, lo:hi, :]

    return {
        "x_bf": x_bf,
        "x_res": x_res,
        "seg3": np.ascontiguousarray(seg3),
        "bvec": _prep_bvec(inp, s == 0, e == H),
        **wshared,
    }


def kernel(**inputs):
    inp = {k: np.asarray(v) for k, v in inputs.items()}

    if "nc" not in _CACHE:
        _CACHE["nc"] = _build()
    nc = _CACHE["nc"]

    wshared = _prep_weights(inp)
    in_maps = [_prep_shard(inp, i, wshared) for i in range(N_CORES)]

    from concourse.bass_utils import run_bass_kernel_spmd
    res = run_bass_kernel_spmd(nc, in_maps, core_ids=list(range(N_CORES)),
                               trace=False)

    out = np.zeros((N, C, H, W), np.float32)
    for i in range(N_CORES):
        n, blk = i // 4, i % 4
        s = RB * blk
        o = res.results[i]["out"]  # (128, 2, 32, 128)
        out[n, :, s:s + RB, :] = o.transpose(1, 0, 2, 3).reshape(C, RB, W)
    return out


# revision 46
# speedup vs baseline: 1.0247x; 1.0081x over previous
"""Trainium2 Bass kernel for nn_DepthsepCCBlock (dense_cnn).

Strategy: 8-way shard over (batch=2) x (H/4 blocks of 32 rows). Each core
computes its 32 output rows end-to-end. The two training-mode BatchNorms
become sync-BN via two tiny (128x4 fp32) AllReduces, both fully overlapped
with TensorEngine work. The dominant 128->2304 3x3 convs run in fp8 e4m3
DoubleRow mode (two taps contracted per PE pass: 9 taps in 5 passes, 1.8x).
Precision is preserved by mean-removal: the conv input is delta = h - c
(c = per-channel mean, pads become -c automatically since h pads are 0) and
the exact-fp32 tap-sum-of-weights times c is folded back in via a per-group
bias computed on device (18 tiny matmuls). This is cell-wise exact for any
c, so only the fp8 quantization of delta and of the weights remains as
error. All other tensors run in fp16 (same PE/DVE speed as bf16, 8x less
noise). The per-pixel dynamic depthwise 3x3 conv runs on the VectorEngine
as 17 tensor-tensor ops per tile, fused with the producing conv. Halo rows
are recomputed from host-sliced zero-padded shards (no halo exchange);
image-boundary BN-bias artifacts are killed with per-core edge-row masks
supplied as data so every core runs one identical NEFF.
"""
import sys
import types
import numpy as np
import ml_dtypes

if "/opt/trn_rl_repo" not in sys.path:
    sys.path.insert(0, "/opt/trn_rl_repo")

F16NP = np.float16
F8NP = ml_dtypes.float8_e4m3

N, C, H, W = 2, 256, 128, 128
SNC, NH = 35, 128
EPS = 1e-5
N_CORES = 8
RB = 32                      # output rows per shard
CNT = float(N * H * W)       # BN reduction count per channel
WSC = 32.0                   # fp8 weight scale

ROWS_B = 34                  # dx_mid rows (s-1 .. e)
ROWS_C = 32                  # output rows (s .. e-1)

# fp8 tap allocation per gw conv: gw1 taps 0-5 via 3 DoubleRow pairs (taps
# 6,7,8 stay fp16), gw2 taps 0-7 via 4 pairs (tap 8 fp16). Chosen from the
# precision sim: (6,8) taps fp8 -> rel err ~1.6e-2 vs 1.84e-2 for (9,9),
# at the cost of one extra PE pass on gw1 only.
GW1_PAIRS = [(0, 1), (2, 3), (4, 5)]
GW1_F16 = [6, 7, 8]
GW2_PAIRS = [(0, 1), (2, 3), (4, 5), (6, 7)]
GW2_F16 = [8]


def _row_tiles(nrows):
    out = []
    r = 0
    while r < nrows:
        nr = min(4, nrows - r)
        out.append((r, nr))
        r += nr
    return out


_CACHE = {}


def _build(level=6):
    from contextlib import ExitStack
    import concourse.tile as tile
    from concourse import bacc, mybir
    from concourse.ap import AP as APc

    F32 = mybir.dt.float32
    F16 = mybir.dt.float16
    F8 = mybir.dt.float8e4
    AF = mybir.ActivationFunctionType
    OP = mybir.AluOpType
    DR = mybir.MatmulPerfMode.DoubleRow

    nc = bacc.Bacc("TRN2", target_bir_lowering=False, debug=False,
                   num_devices=N_CORES)

    x_d = nc.dram_tensor("x_bf", [128, 2, 36, 128], F16, kind="ExternalInput")
    xr_d = nc.dram_tensor("x_res", [128, 2, 32, 128], F32, kind="ExternalInput")
    seg_d = nc.dram_tensor("seg3", [105, 36, 130], F16, kind="ExternalInput")
    w1_d = nc.dram_tensor("w1l", [105, 3, 4, 128], F16, kind="ExternalInput")
    gw1_d = nc.dram_tensor("gw1l", [128, 6, 18, 128], F8, kind="ExternalInput")
    gw2_d = nc.dram_tensor("gw2l", [128, 8, 18, 128], F8, kind="ExternalInput")
    gw1f_d = nc.dram_tensor("gw1f", [128, 3, 18, 128], F16, kind="ExternalInput")
    gw2f_d = nc.dram_tensor("gw2f", [128, 1, 18, 128], F16, kind="ExternalInput")
    ws_d = nc.dram_tensor("wsuml", [128, 2, 18, 128], F16, kind="ExternalInput")
    wse_d = nc.dram_tensor("wssel", [128, 2, 2, 128], F16, kind="ExternalInput")
    se1_d = nc.dram_tensor("se1l", [128, 9, 2, 128], F8, kind="ExternalInput")
    se2_d = nc.dram_tensor("se2l", [128, 9, 2, 128], F8, kind="ExternalInput")
    c1_d = nc.dram_tensor("c1l", [128, 2, 2, 128], F16, kind="ExternalInput")
    c3_d = nc.dram_tensor("c3l", [128, 2, 2, 128], F16, kind="ExternalInput")
    bv_d = nc.dram_tensor("bvec", [128, 58], F32, kind="ExternalInput")
    out_d = nc.dram_tensor("out", [128, 2, 32, 128], F32, kind="ExternalOutput")

    # bvec columns
    B1C = 0          # 4: first-conv biases (h1, h2, h1se, h2se order: gw1,gw2,se1,se2)
    GB1 = 4          # 18: gw1_b2 per (t,half)
    GB2 = 22         # 18: gw2_b2
    SB1 = 40         # 2: se1_b2 ; SB2=42 ; C1B=44 ; C3B=46
    SB2, C1B, C3B = 42, 44, 46
    BN1G, BN1B, BN2G, BN2B = 48, 50, 52, 54
    EMSK = 56        # 2: top,bot edge masks

    with tile.TileContext(nc) as tc:
        with ExitStack() as ctx:
            static = ctx.enter_context(tc.tile_pool(name="static", bufs=1))
            cwtp = ctx.enter_context(tc.tile_pool(name="cwtp", bufs=2))
            workp = ctx.enter_context(tc.tile_pool(name="work", bufs=2))
            dramp = ctx.enter_context(tc.tile_pool(name="dramp", bufs=1, space="DRAM"))
            ph = ctx.enter_context(tc.tile_pool(name="ph", bufs=2, space="PSUM"))
            pcw = ctx.enter_context(tc.tile_pool(name="pcw", bufs=4, space="PSUM"))
            pu = ctx.enter_context(tc.tile_pool(name="pu", bufs=2, space="PSUM"))

            # ---- static SBUF tiles ----
            xsb = static.tile([128, 2, 36, 128], F16, tag="wbig2", name="xsb")
            seg = static.tile([105, 36, 130], F16, tag="segc", name="segsb")
            w1 = static.tile([105, 3, 4, 128], F16, tag="w1c", name="w1sb")
            gw1 = static.tile([128, 6, 18, 128], F8, tag="gwA", name="gw1sb")
            gw1f = static.tile([128, 3, 18, 128], F16, tag="gwF", name="gw1fsb")
            ws = static.tile([128, 2, 18, 128], F16, tag="wsc", name="wssb")
            wse = static.tile([128, 2, 2, 128], F16, tag="wsec", name="wsesb")
            se1 = static.tile([128, 9, 2, 128], F8, tag="seC", name="se1sb")
            dq1se = static.tile([128, 36, 130], F8, tag="dqS", name="dq1sesb")
            bt1se = static.tile([128, 2], F32, tag="btsec", name="bt1sesb")
            bt2se = static.tile([128, 2], F32, tag="btse2c", name="bt2sesb")
            c1 = static.tile([128, 2, 2, 128], F16, tag="c1c", name="c1sb")
            c3 = static.tile([128, 2, 2, 128], F16, tag="c3c", name="c3sb")
            bv = static.tile([128, 58], F32, tag="bvc", name="bvsb")
            h1 = static.tile([128, 36, 130], F16, tag="hB", name="h1sb")
            h1se = static.tile([128, 36, 130], F16, tag="hA", name="h1sesb")
            dq1 = static.tile([128, 36, 130], F8, tag="dqA", name="dq1sb")
            xbn = static.tile([128, 2, 36, 130], F16, tag="xbnc", name="xbnsb")
            sw1 = static.tile([128, 2, 34, 128], F16, tag="swc", name="sw1sb")
            dxm = static.tile([128, 2, 34, 130], F16, tag="dxmc", name="dxmsb")
            st1 = static.tile([128, 22], F32, tag="st1c", name="st1sb")
            st2 = static.tile([128, 22], F32, tag="st2c", name="st2sb")
            cv = static.tile([128, 4], F16, tag="cvc", name="cvsb")
            cv32 = static.tile([128, 4], F32, tag="cv32c", name="cv32sb")
            bt1 = static.tile([128, 18], F32, tag="bt1c", name="bt1sb")
            bt2 = static.tile([128, 18], F32, tag="bt2c", name="bt2sb")

            cc1i = dramp.tile([128, 4], F32, name="cc1i")
            cc1o = dramp.tile([128, 4], F32, addr_space="Shared", name="cc1o")
            cc2i = dramp.tile([128, 4], F32, name="cc2i")
            cc2o = dramp.tile([128, 4], F32, addr_space="Shared", name="cc2o")

            sync, ve, act, gp, te = nc.sync, nc.vector, nc.scalar, nc.gpsimd, nc.tensor

            # ---- input DMAs. One queue, critical-first: the first conv needs
            # only seg/w1/bv, and a single in-order queue guarantees the big
            # loads don't steal HBM bandwidth from them (a multi-queue spread
            # measured 23us slower to first matmul).
            sync.dma_start(seg[:], seg_d.ap())
            sync.dma_start(w1[:], w1_d.ap())
            sync.dma_start(bv[:], bv_d.ap())
            sync.dma_start(xsb[:], x_d.ap())
            sync.dma_start(se1[:], se1_d.ap())
            sync.dma_start(ws[:], ws_d.ap())
            sync.dma_start(wse[:], wse_d.ap())
            sync.dma_start(gw1[:], gw1_d.ap())
            sync.dma_start(gw1f[:], gw1f_d.ap())
            sync.dma_start(c1[:], c1_d.ap())
            sync.dma_start(c3[:], c3_d.ap())

            # ---- zero pad cells. Every row and all interior columns of these
            # buffers get written by drains/affines before any read, so only
            # the two pad columns need zeroing.
            def zero_pads(buf):
                if len(buf.shape) == 4:
                    ve.memset(buf[:, :, :, 0], 0.0)
                    ve.memset(buf[:, :, :, 129], 0.0)
                else:
                    ve.memset(buf[:, :, 0], 0.0)
                    ve.memset(buf[:, :, 129], 0.0)

            for buf in (h1, h1se, xbn, dxm):
                zero_pads(buf)

            def bn1_stats():
                # Emitted after the h convs: ScalarE is the drain engine for
                # the first-conv PSUMs, and 15us of stats passes up front
                # stalls the PE behind the 2-bank ph pool.
                scrA = cwtp.tile([128, 32, 128], F16, tag="cwt", name="scrA")
                for hh in range(2):
                    act.activation(scrA[:], xsb[:, hh, 2:34, :], AF.Identity,
                                   accum_out=st1[:, hh:hh + 1])
                    act.activation(scrA[:], xsb[:, hh, 2:34, :], AF.Square,
                                   accum_out=st1[:, 2 + hh:3 + hh])
                # cc bounce DMAs ride the otherwise-idle gpsimd queue so they
                # are not serialized behind the weight loads on sync's queue.
                gp.dma_start(cc1i[:], st1[:, 0:4])
                gp.collective_compute(
                    "AllReduce", OP.add, replica_groups=[list(range(N_CORES))],
                    ins=[cc1i.opt()], outs=[cc1o.opt()])
                gp.dma_start(st1[:, 4:8], cc1o[:])

            def edge_mask(buf_row_ap, mcol):
                ve.tensor_scalar(buf_row_ap, buf_row_ap,
                                 bv[:, EMSK + mcol:EMSK + mcol + 1], None,
                                 op0=OP.mult)

            # ---- first convs: h = relu(conv(segmap)+b), K=105 (3 sy packed) ----
            def hconv(hbuf, cidx, jof, nrows, mrows):
                for (r0, nr) in _row_tiles(nrows):
                    npx = nr * 128
                    ps = ph.tile([128, 512], F32, tag="ph", name=f"ps_h{cidx}_{r0}")
                    for sx in range(3):
                        te.matmul(ps[:, :npx], w1[:, sx, cidx, :],
                                  seg[:, jof + r0: jof + r0 + nr, sx:sx + 128],
                                  start=(sx == 0), stop=(sx == 2))
                    act.activation(hbuf[:, r0:r0 + nr, 1:129], ps[:, :npx],
                                   AF.Relu, bias=bv[:, B1C + cidx:B1C + cidx + 1])
                edge_mask(hbuf[:, mrows[0], :], 0)
                edge_mask(hbuf[:, mrows[1], :], 1)

            # ---- mean-removal: c = mean(h); dq = h - c (fp8, pads -> -c);
            # bias_g = (Wsum^T c)_g + b2_g via ng tiny matmuls ----
            # scratch columns for the per-h sums (consumed by the DVE mul
            # before bn_coeffs overwrites them; DVE is in-order)
            MR_STC = {0: (st1, 8), 1: (st2, 8), 2: (st1, 10), 3: (st2, 10)}

            def mean_stats(hbuf, dqbuf, ccol, nrows):
                # ScalarE/DVE only: emitted right after the h conv so it runs
                # while the PE continues with the next conv.
                scr = cwtp.tile([128, 36, 130], F16, tag="cwt", name=f"scrc{ccol}")
                stb, sc0 = MR_STC[ccol]
                act.activation(scr[:, :nrows, :], hbuf[:, :nrows, :], AF.Identity,
                               accum_out=stb[:, sc0:sc0 + 1])
                # c is materialized in fp16 (for the fp16 bias matvec) and
                # upcast to an exact fp32 twin for the DVE subtract, so both
                # consumers see bit-identical c and the correction is exact.
                ve.tensor_scalar_mul(cv[:, ccol:ccol + 1], stb[:, sc0:sc0 + 1],
                                     1.0 / (nrows * 130.0))
                act.activation(cv32[:, ccol:ccol + 1], cv[:, ccol:ccol + 1],
                               AF.Identity)
                ve.tensor_scalar(dqbuf[:, :nrows, :], hbuf[:, :nrows, :],
                                 cv32[:, ccol:ccol + 1], None, op0=OP.subtract)

            def mean_bias(ccol, wstile, wsj, btbuf, b2c0, ng):
                # PE matvec: emitted late enough that c is already computed,
                # so the in-order PE never blocks on the stats chain.
                # One accumulation group writing disjoint columns: start only
                # on g=0 (start marks the whole 2KB PSUM bank pending-zero;
                # later columns first-touch-zero their own bytes).
                psb = ph.tile([128, ng], F32, tag="ph", name=f"psb{ccol}")
                for g in range(ng):
                    te.matmul(psb[:, g:g + 1], wstile[:, wsj, g, :],
                              cv[:, ccol:ccol + 1], start=(g == 0),
                              stop=(g == ng - 1), skip_group_check=True)
                ve.tensor_add(btbuf[:], psb[:], bv[:, b2c0:b2c0 + ng])

            # ---- se conv: sw = sigmoid(conv3x3+b), fp8 DoubleRow on the
            # mean-removed dq (4 pairs + 1 single, scale-32 weights) ----
            def seconv(swbuf, dqsrc, sesb, btse, nrows):
                for (r0, nr) in _row_tiles(nrows):
                    npx = nr * 128
                    for hh in range(2):
                        ps = pcw.tile([128, 512], F32, tag="pcw",
                                      name=f"ps_se{nrows}_{r0}_{hh}")
                        for pi, (pa, pb) in enumerate(GW2_PAIRS):
                            sy0, sx0 = divmod(pa, 3)
                            sy1, sx1 = divmod(pb, 3)
                            a = dqsrc[:, r0 + sy0: r0 + sy0 + nr, sx0:sx0 + 128]
                            delta = (sy1 - sy0) * 130 + (sx1 - sx0)
                            rhs = APc(a.tensor, a.offset,
                                      [list(a.ap[0]), [delta, 2], [130, nr],
                                       [1, 128]])
                            te.matmul(ps[:, :npx], sesb[:, pa:pa + 2, hh, :], rhs,
                                      start=(pi == 0), stop=False, perf_mode=DR,
                                      skip_group_check=True)
                        te.matmul(ps[:, :npx], sesb[:, 8, hh, :],
                                  dqsrc[:, r0 + 2: r0 + 2 + nr, 2:2 + 128],
                                  start=False, stop=True, skip_group_check=True)
                        act.activation(swbuf[:, hh, r0:r0 + nr, :], ps[:, :npx],
                                       AF.Sigmoid, scale=1.0 / WSC,
                                       bias=btse[:, hh:hh + 1])

            h2 = static.tile([128, 34, 130], F16, tag="hA", name="h2sb")
            dq2 = static.tile([128, 34, 130], F8, tag="dqA", name="dq2sb")
            if level >= 2:
                hconv(h1, 0, 0, 36, (1, 34))
                # stats pass runs on ScalarE while the PE continues with
                # h1se's convs, so the bias matvec mostly doesn't stall
                if level >= 3.1:
                    mean_stats(h1, dq1, 0, 36)
                    mean_bias(0, ws, 0, bt1, GB1, 18)
                hconv(h1se, 2, 0, 36, (1, 34))
                if level >= 3.1:
                    mean_stats(h1se, dq1se, 2, 36)
                    mean_bias(2, wse, 0, bt1se, SB1, 2)
            bn1_stats()
            if level >= 3:
                seconv(sw1, dq1se, se1, bt1se, 34)

            if level >= 5:
                # h2 computed early: its slot (h1se's) frees after the sw1
                # conv, this PE work extends the window that hides the BN1
                # AllReduce, and it removes the h2 stall at the B->C boundary.
                zero_pads(h2)
                hconv(h2, 1, 1, 34, (0, 33))
                mean_stats(h2, dq2, 1, 34)
                mean_bias(1, ws, 1, bt2, GB2, 18)

            # ---- BN coefficient computation (tiny [128,2] ops) ----
            def bn_coeffs(st, gcol, bcol):
                ve.tensor_scalar_mul(st[:, 8:10], st[:, 4:6], 1.0 / CNT)     # mu
                ve.tensor_scalar_mul(st[:, 10:12], st[:, 6:8], 1.0 / CNT)    # q
                ve.tensor_mul(st[:, 12:14], st[:, 8:10], st[:, 8:10])        # mu^2
                ve.tensor_sub(st[:, 12:14], st[:, 10:12], st[:, 12:14])      # var
                ve.tensor_scalar_add(st[:, 12:14], st[:, 12:14], EPS)        # +eps
                act.activation(st[:, 14:16], st[:, 12:14], AF.Sqrt)
                ve.reciprocal(st[:, 16:18], st[:, 14:16])                    # rstd
                ve.tensor_mul(st[:, 18:20], bv[:, gcol:gcol + 2], st[:, 16:18])  # a
                ve.tensor_mul(st[:, 20:22], st[:, 8:10], st[:, 18:20])
                ve.tensor_sub(st[:, 20:22], bv[:, bcol:bcol + 2], st[:, 20:22])  # b
            # NOTE: bn stats sums land in st[:,4:8] post-allreduce; mean_remove
            # uses st1[:,8:9]/st2[:,8:9] as scratch for the h sums BEFORE
            # bn_coeffs overwrites st[:,8:10]; ordering below guarantees the
            # c values are consumed (cast+bias matmuls) before bn_coeffs runs.

            if level >= 3:
                bn_coeffs(st1, BN1G, BN1B)
                # xbn = a1*x + bb1 (interior cols), then edge-row masks.
                # One half on ScalarE, one on DVE to halve the latency.
                act.activation(xbn[:, 0, :, 1:129], xsb[:, 0, :, :],
                               AF.Identity, scale=st1[:, 18:19],
                               bias=st1[:, 20:21])
                ve.tensor_scalar(xbn[:, 1, :, 1:129], xsb[:, 1, :, :],
                                 st1[:, 19:20], st1[:, 21:22],
                                 op0=OP.mult, op1=OP.add)
                edge_mask(xbn[:, 0, 1, :], 0)
                edge_mask(xbn[:, 1, 1, :], 0)
                edge_mask(xbn[:, 0, 34, :], 1)
                edge_mask(xbn[:, 1, 34, :], 1)

            # ---- fused half-block: conv2(gw) -> depthwise -> 1x1 -> gate ----
            LAG = 1

            def gw_matmuls(ps, npx, gwsb, gwf, pairs, f16taps, dqsrc, hsrc,
                           r0, nr, g):
                # DoubleRow fp8 passes (paired taps on the mean-removed dq)
                # then plain fp16 passes (leftover taps on zero-padded h).
                for pi, (pa, pb) in enumerate(pairs):
                    sy0, sx0 = divmod(pa, 3)
                    sy1, sx1 = divmod(pb, 3)
                    a = dqsrc[:, r0 + sy0: r0 + sy0 + nr, sx0:sx0 + 128]
                    delta = (sy1 - sy0) * 130 + (sx1 - sx0)
                    rhs = APc(a.tensor, a.offset,
                              [list(a.ap[0]), [delta, 2], [130, nr], [1, 128]])
                    te.matmul(ps[:, :npx], gwsb[:, pa:pa + 2, g, :], rhs,
                              start=(pi == 0), stop=False, perf_mode=DR,
                              skip_group_check=True)
                for fi, t in enumerate(f16taps):
                    sy, sx = divmod(t, 3)
                    te.matmul(ps[:, :npx], gwf[:, fi, g, :],
                              hsrc[:, r0 + sy: r0 + sy + nr, sx:sx + 128],
                              start=False, stop=(fi == len(f16taps) - 1),
                              skip_group_check=True)

            def halfblock(first, gwsb, gwf, pairs, f16taps, dqsrc, hsrc, xsrc,
                          btbuf, csb, cbc, swsrc, swrows,
                          do_dc=True, do_flush=True, defer_tail=False,
                          on_flush=None, after_tile0=None):
                tiles = _row_tiles(ROWS_B if first else ROWS_C)
                if not first:
                    # split the final tile so the end-of-kernel pipeline drain
                    # (depthconv + 1x1 + gate + residual) is half as deep
                    (r0l, _) = tiles[-1]
                    tiles = tiles[:-1] + [(r0l, 2), (r0l + 2, 2)]
                pend = []

                def flush_one(item):
                    idx, r0, nr, dps = item
                    npx = nr * 128
                    if not do_flush:
                        return
                    for hh in range(2):
                        up = pu.tile([128, 512], F32, tag="pu",
                                     name=f"up{int(first)}_{idx}_{hh}")
                        te.matmul(up[:, :npx], csb[:, 0, hh, :], dps[0][:, :nr, :],
                                  start=True, stop=False)
                        te.matmul(up[:, :npx], csb[:, 1, hh, :], dps[1][:, :nr, :],
                                  start=False, stop=True)
                        z = workp.tile([128, 512], F16, tag="z", bufs=1,
                                       name=f"z{int(first)}_{idx}_{hh}")
                        ve.scalar_tensor_tensor(
                            z[:, :npx], up[:, :npx], bv[:, cbc + hh:cbc + hh + 1],
                            swsrc[:, hh, r0:r0 + nr, :],
                            op0=OP.add, op1=OP.mult)
                        if first:
                            ve.scalar_tensor_tensor(
                                dxm[:, hh, r0:r0 + nr, 1:129], z[:, :npx], 0.2,
                                z[:, :npx], op0=OP.mult, op1=OP.max)
                        else:
                            d2 = workp.tile([128, 512], F16, tag="d2", bufs=1,
                                            name=f"d2_{idx}_{hh}")
                            ve.scalar_tensor_tensor(
                                d2[:, :npx], z[:, :npx], 0.2, z[:, :npx],
                                op0=OP.mult, op1=OP.max)
                            xrt = workp.tile([128, 512], F32, tag="xr",
                                             name=f"xr_{idx}_{hh}")
                            sync.dma_start(xrt[:, :npx], xr_d.ap()[:, hh, r0:r0 + nr, :])
                            stg = workp.tile([128, 512], F32, tag="st",
                                             name=f"st_{idx}_{hh}")
                            ve.tensor_add(stg[:, :npx], d2[:, :npx], xrt[:, :npx])
                            sync.dma_start(out_d.ap()[:, hh, r0:r0 + nr, :], stg[:, :npx])
                    if on_flush is not None:
                        on_flush(idx, r0, nr)

                for idx, (r0, nr) in enumerate(tiles):
                    npx = nr * 128
                    cwt = cwtp.tile([128, 18, 512], F16, tag="cwt",
                                    name=f"cwt{int(first)}_{idx}")
                    for g in range(18):
                        ps = pcw.tile([128, 512], F32, tag="pcw",
                                      name=f"pcw{int(first)}_{idx}_{g}")
                        gw_matmuls(ps, npx, gwsb, gwf, pairs, f16taps,
                                   dqsrc, hsrc, r0, nr, g)
                        act.activation(cwt[:, g, :npx], ps[:, :npx],
                                       AF.Identity, scale=1.0 / WSC,
                                       bias=btbuf[:, g:g + 1])
                    if idx == 0 and after_tile0 is not None:
                        # AR-independent PE work emitted before the first
                        # flush: covers the BN2-AllReduce wait at the B->C
                        # boundary (the in-order PE would otherwise stall at
                        # the first 1x1, which depends on bn2dx)
                        after_tile0()
                    dps = []
                    for hh in range(2 if do_dc else 0):
                        dp = workp.tile([128, 4, 128], F16, tag=f"dp{hh}",
                                        name=f"dp{int(first)}_{idx}_{hh}")
                        tmp = workp.tile([128, 512], F16, tag="tmp", bufs=1,
                                         name=f"tmp{int(first)}_{idx}_{hh}")
                        for t in range(9):
                            ty, tx = divmod(t, 3)
                            src = xsrc[:, hh, r0 + ty: r0 + ty + nr, tx:tx + 128]
                            if t == 0:
                                ve.tensor_mul(dp[:, :nr, :], cwt[:, hh, :npx], src)
                            else:
                                ve.tensor_mul(tmp[:, :npx], cwt[:, t * 2 + hh, :npx], src)
                                ve.tensor_add(dp[:, :nr, :], dp[:, :nr, :], tmp[:, :npx])
                        dps.append(dp)
                    pend.append((idx, r0, nr, dps))
                    if len(pend) > LAG:
                        flush_one(pend.pop(0))
                if defer_tail:
                    return pend, flush_one
                while pend:
                    flush_one(pend.pop(0))
                return None

            if level < 6:
                bstg = static.tile([128, 128], F32, tag="bstg", name="bstg")
                ve.memset(bstg[:], 0.0)
                sync.dma_start(out_d.ap()[:, 0, 0, :], bstg[:])

            # incremental BN2 stats: accumulate partial sums per flushed dxm
            # chunk so the AllReduce can start right after the last flush
            # instead of after four full-height stats passes.
            stp = static.tile([128, 4, 3], F32, tag="stpc", name="stpsb")
            BN2_CHUNKS = {2: (0, 1, 12), 5: (1, 12, 24), 8: (2, 24, 33)}

            def bn2_inc(idx, r0, nr):
                if level < 3.8 or idx not in BN2_CHUNKS:
                    return
                ci, lo, hi = BN2_CHUNKS[idx]
                for hh in range(2):
                    scr = cwtp.tile([128, 12, 130], F16, tag="cwt",
                                    name=f"scrB{ci}_{hh}")
                    nrw = hi - lo
                    act.activation(scr[:, :nrw, :], dxm[:, hh, lo:hi, :],
                                   AF.Identity, accum_out=stp[:, hh, ci:ci + 1])
                    act.activation(scr[:, :nrw, :], dxm[:, hh, lo:hi, :],
                                   AF.Square, accum_out=stp[:, 2 + hh, ci:ci + 1])

            tailB = None
            if level >= 3.2:
                tailB = halfblock(True, gw1, gw1f, GW1_PAIRS, GW1_F16, dq1, h1,
                                  xbn, bt1, c1, C1B, sw1, ROWS_B,
                                  do_dc=(level >= 3.4), do_flush=(level >= 3.6),
                                  defer_tail=(level >= 6),
                                  on_flush=bn2_inc if level >= 3.6 else None)

            if level >= 5:
                # Emit phase-C fmap convs before draining B's tail flushes so
                # the PE has independent work while the DVE finishes phase B.
                gw2 = static.tile([128, 8, 18, 128], F8, tag="gwA", name="gw2sb")
                sync.dma_start(gw2[:], gw2_d.ap())
                gw2f = static.tile([128, 1, 18, 128], F16, tag="gwF", name="gw2fsb")
                sync.dma_start(gw2f[:], gw2f_d.ap())
                se2 = static.tile([128, 9, 2, 128], F8, tag="seC", name="se2sb")
                sync.dma_start(se2[:], se2_d.ap())
                h2se = static.tile([128, 34, 130], F16, tag="hB", name="h2sesb")
                dq2se = static.tile([128, 34, 130], F8, tag="dqS", name="dq2sesb")
                sw2 = static.tile([128, 2, 32, 128], F16, tag="swc", name="sw2sb")
                bn2dx = static.tile([128, 2, 34, 130], F16, tag="wbig2",
                                    name="bn2dxsb")
                zero_pads(h2se)
                zero_pads(bn2dx)
                hconv(h2se, 3, 1, 34, (0, 33))
                mean_stats(h2se, dq2se, 3, 34)
                mean_bias(3, wse, 1, bt2se, SB2, 2)
                seconv(sw2, dq2se, se2, bt2se, 32)

            if tailB is not None:
                pendB, flushB = tailB
                while pendB:
                    flushB(pendB.pop(0))

            if level >= 3.8:
                # reduce the 3 per-chunk partial sums into st2[:, 0:4]
                # (layout stp[128, stat j, chunk]: j = {sum_h0,sum_h1,sq_h0,sq_h1})
                ve.tensor_add(st2[:, 0:4], stp[:, :, 0], stp[:, :, 1])
                ve.tensor_add(st2[:, 0:4], st2[:, 0:4], stp[:, :, 2])
            if level >= 4:
                gp.dma_start(cc2i[:], st2[:, 0:4])
                gp.collective_compute(
                    "AllReduce", OP.add, replica_groups=[list(range(N_CORES))],
                    ins=[cc2i.opt()], outs=[cc2o.opt()])
                gp.dma_start(st2[:, 4:8], cc2o[:])

            if level >= 5:
                bn_coeffs(st2, BN2G, BN2B)
                # one half ScalarE, one DVE: halves the post-AllReduce latency
                act.activation(bn2dx[:, 0, :, 1:129], dxm[:, 0, :, 1:129],
                               AF.Identity, scale=st2[:, 18:19],
                               bias=st2[:, 20:21])
                ve.tensor_scalar(bn2dx[:, 1, :, 1:129], dxm[:, 1, :, 1:129],
                                 st2[:, 19:20], st2[:, 21:22],
                                 op0=OP.mult, op1=OP.add)
                edge_mask(bn2dx[:, 0, 0, :], 0)
                edge_mask(bn2dx[:, 1, 0, :], 0)
                edge_mask(bn2dx[:, 0, 33, :], 1)
                edge_mask(bn2dx[:, 1, 33, :], 1)

            if level >= 6:
                halfblock(False, gw2, gw2f, GW2_PAIRS, GW2_F16, dq2, h2,
                          bn2dx, bt2, c3, C3B, sw2, ROWS_C)

    nc.compile()
    return nc


# ---------------------------------------------------------------------------
# Host-side sharding / layout prep
# ---------------------------------------------------------------------------

def _prep_weights(inp):
    """Shared (shard-independent) weight/bias layout prep."""
    def f16a(a):
        return np.ascontiguousarray(a.astype(F16NP))

    out = {}
    # first convs, K=105 (sy*35+cin), per sx, per conv j in (gw1, gw2, se1, se2)
    w1l = np.zeros((105, 3, 4, 128), np.float32)
    for j, wkey in enumerate(("gw1_w1", "gw2_w1", "se1_w1", "se2_w1")):
        wj = inp[wkey]  # (128, 35, 3, 3)
        for sy in range(3):
            for sx in range(3):
                w1l[sy * 35:(sy + 1) * 35, sx, j, :] = wj[:, :, sy, sx].T
    out["w1l"] = f16a(w1l)

    m = np.arange(128)
    wsum = np.zeros((128, 2, 18, 128), np.float32)
    n8 = {0: 6, 1: 8}  # fp8 tap count per conv (rest go to the fp16 tensor)
    for jj, (key, fkey, src) in enumerate((("gw1l", "gw1f", "gw1_w2"),
                                           ("gw2l", "gw2f", "gw2_w2"))):
        wsrc = inp[src]  # (2304, 128, 3, 3)
        gl = np.zeros((128, 9, 18, 128), np.float32)
        for s9 in range(9):
            sy, sx = divmod(s9, 3)
            for g in range(18):
                t, hh = g // 2, g % 2
                rows = (hh * 128 + m) * 9 + t
                gl[:, s9, g, :] = wsrc[rows, :, sy, sx].T
        k = n8[jj]
        # c-trick bias uses the exact fp32 tap-sum of the fp8 taps only
        wsum[:, jj, :, :] = gl[:, :k].sum(axis=1)
        out[key] = np.ascontiguousarray((gl[:, :k] * WSC).astype(F8NP))
        # fp16 leftover taps pre-scaled by 32 (power of two: exact in fp16)
        # so the uniform 1/32 drain scale applies to the whole PSUM
        out[fkey] = np.ascontiguousarray((gl[:, k:] * WSC).astype(F16NP))
    out["wsuml"] = f16a(wsum)

    wse = np.zeros((128, 2, 2, 128), np.float32)
    for jj, (key, src) in enumerate((("se1l", "se1_w2"), ("se2l", "se2_w2"))):
        wsrc = inp[src]  # (256, 128, 3, 3)
        sl = np.zeros((128, 9, 2, 128), np.float32)
        for s9 in range(9):
            sy, sx = divmod(s9, 3)
            for hh in range(2):
                sl[:, s9, hh, :] = wsrc[hh * 128 + m, :, sy, sx].T
        wse[:, jj, :, :] = sl.sum(axis=1)
        out[key] = np.ascontiguousarray((sl * WSC).astype(F8NP))
    out["wssel"] = f16a(wse)

    for key, src in (("c1l", "conv1_w"), ("c3l", "conv3_w")):
        wsrc = inp[src][:, :, 0, 0]  # (256, 256) [cout, cin]
        cl = np.zeros((128, 2, 2, 128), np.float32)
        for kg in range(2):
            for hh in range(2):
                # cl[k, kg, hh, mo] = w[hh*128+mo, kg*128+k]
                cl[:, kg, hh, :] = wsrc[hh * 128:(hh + 1) * 128,
                                        kg * 128:(kg + 1) * 128].T
        out[key] = f16a(cl)
    return out


def _prep_bvec(inp, top_edge, bot_edge):
    bvec = np.zeros((128, 58), np.float32)
    for j, k in enumerate(("gw1_b1", "gw2_b1", "se1_b1", "se2_b1")):
        bvec[:, j] = inp[k]
    for base, k in ((4, "gw1_b2"), (22, "gw2_b2")):
        b2 = inp[k]
        for g in range(18):
            t, hh = g // 2, g % 2
            bvec[:, base + g] = b2[(hh * 128 + np.arange(128)) * 9 + t]
    for base, k in ((40, "se1_b2"), (42, "se2_b2"), (44, "conv1_b"), (46, "conv3_b"),
                    (48, "bn1_g"), (50, "bn1_b"), (52, "bn2_g"), (54, "bn2_b")):
        v = inp[k]
        bvec[:, base] = v[:128]
        bvec[:, base + 1] = v[128:]
    bvec[:, 56] = 0.0 if top_edge else 1.0
    bvec[:, 57] = 0.0 if bot_edge else 1.0
    return bvec


def _prep_shard(inp, i, wshared):
    n, blk = i // 4, i % 4
    s = RB * blk
    e = s + RB
    x = inp["x"][n]                       # (256,128,128)
    seg_ds = inp["seg"][n][:, ::2, ::2]   # (35,128,128)

    xg = x.reshape(2, 128, H, W)

    x_bf = np.zeros((128, 2, 36, 128), F16NP)
    lo, hi = max(s - 2, 0), min(e + 2, H)
    x_bf[:, :, lo - (s - 2):hi - (s - 2), :] = \
        xg[:, :, lo:hi, :].transpose(1, 0, 2, 3).astype(F16NP)

    x_res = np.ascontiguousarray(
        xg[:, :, s:e, :].transpose(1, 0, 2, 3).astype(np.float32))

    seg3 = np.zeros((105, 36, 130), F16NP)
    seg_f = seg_ds.astype(F16NP)
    for sy in range(3):
        # seg3[sy*35+c, j, 1+x] = segmap[c, s-3+j+sy, x]
        r0 = s - 3 + sy
        lo, hi = max(r0, 0), min(r0 + 36, H)
        if hi > lo:
            seg3[sy * 35:(sy + 1) * 35, lo - r0:hi - r0, 1:129] = seg_f[:, lo:hi, :]

    return {
        "x_bf": x_bf,
        "x_res": x_res,
        "seg3": np.ascontiguousarray(seg3),
        "bvec": _prep_bvec(inp, s == 0, e == H),
        **wshared,
    }


def kernel(**inputs):
    inp = {k: np.asarray(v) for k, v in inputs.items()}

    if "nc" not in _CACHE:
        _CACHE["nc"] = _build()
    nc = _CACHE["nc"]

    wshared = _prep_weights(inp)
    in_maps = [_prep_shard(inp, i, wshared) for i in range(N_CORES)]

    from concourse.bass_utils import run_bass_kernel_spmd
    res = run_bass_kernel_spmd(nc, in_maps, core_ids=list(range(N_CORES)),
                               trace=False)

    out = np.zeros((N, C, H, W), np.float32)
    for i in range(N_CORES):
        n, blk = i // 4, i % 4
        s = RB * blk
        o = res.results[i]["out"]  # (128, 2, 32, 128)
        out[n, :, s:s + RB, :] = o.transpose(1, 0, 2, 3).reshape(C, RB, W)
    return out


# revision 55
# speedup vs baseline: 1.0410x; 1.0160x over previous
"""Trainium2 Bass kernel for nn_DepthsepCCBlock (dense_cnn).

Strategy: 8-way shard over (batch=2) x (H/4 blocks of 32 rows). Each core
computes its 32 output rows end-to-end. The two training-mode BatchNorms
become sync-BN via two tiny (128x4 fp32) AllReduces, both fully overlapped
with TensorEngine work. The dominant 128->2304 3x3 convs run in fp8 e4m3
DoubleRow mode (two taps contracted per PE pass: 9 taps in 5 passes, 1.8x).
Precision is preserved by mean-removal: the conv input is delta = h - c
(c = per-channel mean, pads become -c automatically since h pads are 0) and
the exact-fp32 tap-sum-of-weights times c is folded back in via a per-group
bias computed on device (18 tiny matmuls). This is cell-wise exact for any
c, so only the fp8 quantization of delta and of the weights remains as
error. All other tensors run in fp16 (same PE/DVE speed as bf16, 8x less
noise). The per-pixel dynamic depthwise 3x3 conv runs on the VectorEngine
as 17 tensor-tensor ops per tile, fused with the producing conv. Halo rows
are recomputed from host-sliced zero-padded shards (no halo exchange);
image-boundary BN-bias artifacts are killed with per-core edge-row masks
supplied as data so every core runs one identical NEFF.
"""
import sys
import numpy as np
import ml_dtypes

if "/opt/trn_rl_repo" not in sys.path:
    sys.path.insert(0, "/opt/trn_rl_repo")

F16NP = np.float16
F8NP = ml_dtypes.float8_e4m3

N, C, H, W = 2, 256, 128, 128
SNC, NH = 35, 128
EPS = 1e-5
N_CORES = 8
RB = 32                      # output rows per shard
CNT = float(N * H * W)       # BN reduction count per channel
WSC = 32.0                   # fp8 weight scale

ROWS_B = 34                  # dx_mid rows (s-1 .. e)
ROWS_C = 32                  # output rows (s .. e-1)

# fp8 tap allocation per gw conv: gw1 taps 0-5 via 3 DoubleRow pairs (taps
# 6,7,8 stay fp16), gw2 taps 0-7 via 4 pairs (tap 8 fp16). Chosen from the
# precision sim: (6,8) taps fp8 -> rel err ~1.6e-2 vs 1.84e-2 for (9,9),
# at the cost of one extra PE pass on gw1 only.
GW1_PAIRS = [(0, 1), (2, 3), (4, 5)]
GW1_F16 = [6, 7, 8]
GW2_PAIRS = [(0, 1), (2, 3), (4, 5), (6, 7)]
GW2_F16 = [8]


def _row_tiles(nrows):
    out = []
    r = 0
    while r < nrows:
        nr = min(4, nrows - r)
        out.append((r, nr))
        r += nr
    return out


_CACHE = {}


def _build(level=6):
    from contextlib import ExitStack
    import concourse.tile as tile
    from concourse import bacc, mybir
    from concourse.ap import AP as APc

    F32 = mybir.dt.float32
    F16 = mybir.dt.float16
    F8 = mybir.dt.float8e4
    AF = mybir.ActivationFunctionType
    OP = mybir.AluOpType
    DR = mybir.MatmulPerfMode.DoubleRow

    nc = bacc.Bacc("TRN2", target_bir_lowering=False, debug=False,
                   num_devices=N_CORES)

    x_d = nc.dram_tensor("x_bf", [128, 2, 36, 128], F16, kind="ExternalInput")
    seg_d = nc.dram_tensor("seg3", [105, 36, 130], F16, kind="ExternalInput")
    w1_d = nc.dram_tensor("w1l", [105, 3, 4, 128], F16, kind="ExternalInput")
    gw1_d = nc.dram_tensor("gw1l", [128, 6, 18, 128], F8, kind="ExternalInput")
    gw2_d = nc.dram_tensor("gw2l", [128, 8, 18, 128], F8, kind="ExternalInput")
    gw1f_d = nc.dram_tensor("gw1f", [128, 3, 18, 128], F16, kind="ExternalInput")
    gw2f_d = nc.dram_tensor("gw2f", [128, 1, 18, 128], F16, kind="ExternalInput")
    ws_d = nc.dram_tensor("wsuml", [128, 2, 18, 128], F16, kind="ExternalInput")
    wse_d = nc.dram_tensor("wssel", [128, 2, 2, 128], F16, kind="ExternalInput")
    se1_d = nc.dram_tensor("se1l", [128, 9, 2, 128], F8, kind="ExternalInput")
    se2_d = nc.dram_tensor("se2l", [128, 9, 2, 128], F8, kind="ExternalInput")
    c1_d = nc.dram_tensor("c1l", [128, 2, 2, 128], F16, kind="ExternalInput")
    c3_d = nc.dram_tensor("c3l", [128, 2, 2, 128], F16, kind="ExternalInput")
    bv_d = nc.dram_tensor("bvec", [128, 58], F32, kind="ExternalInput")
    out_d = nc.dram_tensor("out", [128, 2, 32, 128], F32, kind="ExternalOutput")

    # bvec columns
    B1C = 0          # 4: first-conv biases (h1, h2, h1se, h2se order: gw1,gw2,se1,se2)
    GB1 = 4          # 18: gw1_b2 per (t,half)
    GB2 = 22         # 18: gw2_b2
    SB1 = 40         # 2: se1_b2 ; SB2=42 ; C1B=44 ; C3B=46
    SB2, C1B, C3B = 42, 44, 46
    BN1G, BN1B, BN2G, BN2B = 48, 50, 52, 54
    EMSK = 56        # 2: top,bot edge masks

    with tile.TileContext(nc) as tc:
        with ExitStack() as ctx:
            static = ctx.enter_context(tc.tile_pool(name="static", bufs=1))
            cwtp = ctx.enter_context(tc.tile_pool(name="cwtp", bufs=2))
            workp = ctx.enter_context(tc.tile_pool(name="work", bufs=2))
            dramp = ctx.enter_context(tc.tile_pool(name="dramp", bufs=1, space="DRAM"))
            ph = ctx.enter_context(tc.tile_pool(name="ph", bufs=2, space="PSUM"))
            pcw = ctx.enter_context(tc.tile_pool(name="pcw", bufs=4, space="PSUM"))
            pu = ctx.enter_context(tc.tile_pool(name="pu", bufs=2, space="PSUM"))

            # ---- static SBUF tiles ----
            xsb = static.tile([128, 2, 36, 128], F16, tag="wbig2", name="xsb")
            seg = static.tile([105, 36, 130], F16, tag="segc", name="segsb")
            w1 = static.tile([105, 3, 4, 128], F16, tag="w1c", name="w1sb")
            gw1 = static.tile([128, 6, 18, 128], F8, tag="gwA", name="gw1sb")
            gw1f = static.tile([128, 3, 18, 128], F16, tag="gwF", name="gw1fsb")
            ws = static.tile([128, 2, 18, 128], F16, tag="wsc", name="wssb")
            wse = static.tile([128, 2, 2, 128], F16, tag="wsec", name="wsesb")
            se1 = static.tile([128, 9, 2, 128], F8, tag="seC", name="se1sb")
            dq1se = static.tile([128, 36, 130], F8, tag="dqS", name="dq1sesb")
            bt1se = static.tile([128, 2], F32, tag="btsec", name="bt1sesb")
            bt2se = static.tile([128, 2], F32, tag="btse2c", name="bt2sesb")
            c1 = static.tile([128, 2, 2, 128], F16, tag="c1c", name="c1sb")
            c3 = static.tile([128, 2, 2, 128], F16, tag="c3c", name="c3sb")
            bv = static.tile([128, 58], F32, tag="bvc", name="bvsb")
            h1 = static.tile([128, 36, 130], F16, tag="hB", name="h1sb")
            h1se = static.tile([128, 36, 130], F16, tag="hA", name="h1sesb")
            dq1 = static.tile([128, 36, 130], F8, tag="dqA", name="dq1sb")
            xbn = static.tile([128, 2, 36, 130], F16, tag="xbnc", name="xbnsb")
            sw1 = static.tile([128, 2, 34, 128], F16, tag="swc", name="sw1sb")
            dxm = static.tile([128, 2, 34, 130], F16, tag="dxmc", name="dxmsb")
            st1 = static.tile([128, 22], F32, tag="st1c", name="st1sb")
            st2 = static.tile([128, 22], F32, tag="st2c", name="st2sb")
            cv = static.tile([128, 4], F16, tag="cvc", name="cvsb")
            cv32 = static.tile([128, 4], F32, tag="cv32c", name="cv32sb")
            hsum = static.tile([128, 4, 9], F32, tag="hsumc", name="hsumsb")
            bt1 = static.tile([128, 18], F32, tag="bt1c", name="bt1sb")
            bt2 = static.tile([128, 18], F32, tag="bt2c", name="bt2sb")

            cc1i = dramp.tile([128, 4], F32, name="cc1i")
            cc1o = dramp.tile([128, 4], F32, addr_space="Shared", name="cc1o")
            cc2i = dramp.tile([128, 4], F32, name="cc2i")
            cc2o = dramp.tile([128, 4], F32, addr_space="Shared", name="cc2o")

            sync, ve, act, gp, te = nc.sync, nc.vector, nc.scalar, nc.gpsimd, nc.tensor

            # ---- input DMAs. One queue, critical-first: the first conv needs
            # only seg/w1/bv, and a single in-order queue guarantees the big
            # loads don't steal HBM bandwidth from them (a multi-queue spread
            # measured 23us slower to first matmul). The first 12 seg rows
            # (3 conv tiles) ship ahead of the bulk so the PE starts sooner.
            sync.dma_start(seg[:, 0:12, :], seg_d.ap()[:, 0:12, :])
            sync.dma_start(w1[:], w1_d.ap())
            sync.dma_start(bv[:], bv_d.ap())
            sync.dma_start(seg[:, 12:36, :], seg_d.ap()[:, 12:36, :])
            sync.dma_start(xsb[:], x_d.ap())
            sync.dma_start(se1[:], se1_d.ap())
            sync.dma_start(ws[:], ws_d.ap())
            sync.dma_start(wse[:], wse_d.ap())
            sync.dma_start(gw1[:], gw1_d.ap())
            sync.dma_start(gw1f[:], gw1f_d.ap())
            sync.dma_start(c1[:], c1_d.ap())
            sync.dma_start(c3[:], c3_d.ap())

            # ---- zero pad cells. Every row and all interior columns of these
            # buffers get written by drains/affines before any read, so only
            # the two pad columns need zeroing.
            def zero_pads(buf):
                if len(buf.shape) == 4:
                    ve.memset(buf[:, :, :, 0], 0.0)
                    ve.memset(buf[:, :, :, 129], 0.0)
                else:
                    ve.memset(buf[:, :, 0], 0.0)
                    ve.memset(buf[:, :, 129], 0.0)

            for buf in (h1, h1se, xbn, dxm):
                zero_pads(buf)

            def bn1_stats():
                # Emitted after the h convs: ScalarE is the drain engine for
                # the first-conv PSUMs, and 15us of stats passes up front
                # stalls the PE behind the 2-bank ph pool.
                scrA = cwtp.tile([128, 32, 128], F16, tag="cwt", name="scrA")
                for hh in range(2):
                    act.activation(scrA[:], xsb[:, hh, 2:34, :], AF.Identity,
                                   accum_out=st1[:, hh:hh + 1])
                    act.activation(scrA[:], xsb[:, hh, 2:34, :], AF.Square,
                                   accum_out=st1[:, 2 + hh:3 + hh])
                # cc bounce DMAs ride the otherwise-idle gpsimd queue so they
                # are not serialized behind the weight loads on sync's queue.
                gp.dma_start(cc1i[:], st1[:, 0:4])
                gp.collective_compute(
                    "AllReduce", OP.add, replica_groups=[list(range(N_CORES))],
                    ins=[cc1i.opt()], outs=[cc1o.opt()])
                gp.dma_start(st1[:, 4:8], cc1o[:])

            def edge_mask(buf_row_ap, mcol):
                ve.tensor_scalar(buf_row_ap, buf_row_ap,
                                 bv[:, EMSK + mcol:EMSK + mcol + 1], None,
                                 op0=OP.mult)

            # ---- first convs: h = relu(conv(segmap)+b), K=105 (3 sy packed).
            # Each drain also accumulates its tile's h-sum (free on ScalarE):
            # feeds the mean-removal c with no extra stats pass. The sums see
            # pre-edge-mask values, which is fine — the c-trick is exact for
            # any c. ----
            def hconv(hbuf, cidx, jof, nrows, mrows):
                for ti, (r0, nr) in enumerate(_row_tiles(nrows)):
                    npx = nr * 128
                    ps = ph.tile([128, 512], F32, tag="ph", name=f"ps_h{cidx}_{r0}")
                    for sx in range(3):
                        te.matmul(ps[:, :npx], w1[:, sx, cidx, :],
                                  seg[:, jof + r0: jof + r0 + nr, sx:sx + 128],
                                  start=(sx == 0), stop=(sx == 2))
                    act.activation(hbuf[:, r0:r0 + nr, 1:129], ps[:, :npx],
                                   AF.Relu, bias=bv[:, B1C + cidx:B1C + cidx + 1],
                                   accum_out=hsum[:, cidx, ti:ti + 1])
                edge_mask(hbuf[:, mrows[0], :], 0)
                edge_mask(hbuf[:, mrows[1], :], 1)

            # ---- mean-removal: c = mean(h); dq = h - c (fp8, pads -> -c);
            # bias_g = (Wsum^T c)_g + b2_g via ng tiny matmuls ----
            def mean_stats(hbuf, dqbuf, ccol, nrows):
                # DVE-only: tree-reduce the per-tile drain sums into c, then
                # cast dq = h - c. No full-buffer scan needed.
                s = hsum[:, ccol, :]
                n = 9
                while n > 1:
                    hn = n // 2
                    ve.tensor_add(s[:, 0:hn], s[:, 0:hn], s[:, n - hn:n])
                    n = n - hn
                # c is materialized in fp16 (for the fp16 bias matvec) and
                # upcast to an exact fp32 twin for the DVE subtract, so both
                # consumers see bit-identical c and the correction is exact.
                ve.tensor_scalar_mul(cv[:, ccol:ccol + 1], s[:, 0:1],
                                     1.0 / (nrows * 130.0))
                act.activation(cv32[:, ccol:ccol + 1], cv[:, ccol:ccol + 1],
                               AF.Identity)
                ve.tensor_scalar(dqbuf[:, :nrows, :], hbuf[:, :nrows, :],
                                 cv32[:, ccol:ccol + 1], None, op0=OP.subtract)

            def mean_bias(ccol, wstile, wsj, btbuf, b2c0, ng):
                # PE matvec: emitted late enough that c is already computed,
                # so the in-order PE never blocks on the stats chain.
                # One accumulation group writing disjoint columns: start only
                # on g=0 (start marks the whole 2KB PSUM bank pending-zero;
                # later columns first-touch-zero their own bytes).
                psb = ph.tile([128, ng], F32, tag="ph", name=f"psb{ccol}")
                for g in range(ng):
                    te.matmul(psb[:, g:g + 1], wstile[:, wsj, g, :],
                              cv[:, ccol:ccol + 1], start=(g == 0),
                              stop=(g == ng - 1), skip_group_check=True)
                ve.tensor_add(btbuf[:], psb[:], bv[:, b2c0:b2c0 + ng])

            # ---- se conv: sw = sigmoid(conv3x3+b), fp8 DoubleRow on the
            # mean-removed dq (4 pairs + 1 single, scale-32 weights) ----
            def seconv(swbuf, dqsrc, sesb, btse, nrows):
                for (r0, nr) in _row_tiles(nrows):
                    npx = nr * 128
                    for hh in range(2):
                        ps = pcw.tile([128, 512], F32, tag="pcw",
                                      name=f"ps_se{nrows}_{r0}_{hh}")
                        for pi, (pa, pb) in enumerate(GW2_PAIRS):
                            sy0, sx0 = divmod(pa, 3)
                            sy1, sx1 = divmod(pb, 3)
                            a = dqsrc[:, r0 + sy0: r0 + sy0 + nr, sx0:sx0 + 128]
                            delta = (sy1 - sy0) * 130 + (sx1 - sx0)
                            rhs = APc(a.tensor, a.offset,
                                      [list(a.ap[0]), [delta, 2], [130, nr],
                                       [1, 128]])
                            te.matmul(ps[:, :npx], sesb[:, pa:pa + 2, hh, :], rhs,
                                      start=(pi == 0), stop=False, perf_mode=DR,
                                      skip_group_check=True)
                        te.matmul(ps[:, :npx], sesb[:, 8, hh, :],
                                  dqsrc[:, r0 + 2: r0 + 2 + nr, 2:2 + 128],
                                  start=False, stop=True, skip_group_check=True)
                        act.activation(swbuf[:, hh, r0:r0 + nr, :], ps[:, :npx],
                                       AF.Sigmoid, scale=1.0 / WSC,
                                       bias=btse[:, hh:hh + 1])

            h2 = static.tile([128, 34, 130], F16, tag="hA", name="h2sb")
            dq2 = static.tile([128, 34, 130], F8, tag="dqA", name="dq2sb")
            if level >= 2:
                hconv(h1, 0, 0, 36, (1, 34))
                # stats pass runs on ScalarE while the PE continues with
                # h1se's convs, so the bias matvec mostly doesn't stall
                if level >= 3.1:
                    mean_stats(h1, dq1, 0, 36)
                    mean_bias(0, ws, 0, bt1, GB1, 18)
                hconv(h1se, 2, 0, 36, (1, 34))
                if level >= 3.1:
                    mean_stats(h1se, dq1se, 2, 36)
                    mean_bias(2, wse, 0, bt1se, SB1, 2)
            bn1_stats()
            if level >= 3:
                seconv(sw1, dq1se, se1, bt1se, 34)

            if level >= 5:
                # h2 computed early: its slot (h1se's) frees after the sw1
                # conv, this PE work extends the window that hides the BN1
                # AllReduce, and it removes the h2 stall at the B->C boundary.
                zero_pads(h2)
                hconv(h2, 1, 1, 34, (0, 33))
                mean_stats(h2, dq2, 1, 34)
                mean_bias(1, ws, 1, bt2, GB2, 18)

            # ---- BN coefficient computation (tiny [128,2] ops) ----
            def bn_coeffs(st, gcol, bcol):
                ve.tensor_scalar_mul(st[:, 8:10], st[:, 4:6], 1.0 / CNT)     # mu
                ve.tensor_scalar_mul(st[:, 10:12], st[:, 6:8], 1.0 / CNT)    # q
                ve.tensor_mul(st[:, 12:14], st[:, 8:10], st[:, 8:10])        # mu^2
                ve.tensor_sub(st[:, 12:14], st[:, 10:12], st[:, 12:14])      # var
                ve.tensor_scalar_add(st[:, 12:14], st[:, 12:14], EPS)        # +eps
                act.activation(st[:, 14:16], st[:, 12:14], AF.Sqrt)
                ve.reciprocal(st[:, 16:18], st[:, 14:16])                    # rstd
                ve.tensor_mul(st[:, 18:20], bv[:, gcol:gcol + 2], st[:, 16:18])  # a
                ve.tensor_mul(st[:, 20:22], st[:, 8:10], st[:, 18:20])
                ve.tensor_sub(st[:, 20:22], bv[:, bcol:bcol + 2], st[:, 20:22])  # b
            # NOTE: bn stats sums land in st[:,4:8] post-allreduce; mean_remove
            # uses st1[:,8:9]/st2[:,8:9] as scratch for the h sums BEFORE
            # bn_coeffs overwrites st[:,8:10]; ordering below guarantees the
            # c values are consumed (cast+bias matmuls) before bn_coeffs runs.

            if level >= 3:
                bn_coeffs(st1, BN1G, BN1B)
                # xbn = a1*x + bb1 (interior cols), then edge-row masks.
                # One half on ScalarE, one on DVE to halve the latency.
                act.activation(xbn[:, 0, :, 1:129], xsb[:, 0, :, :],
                               AF.Identity, scale=st1[:, 18:19],
                               bias=st1[:, 20:21])
                ve.tensor_scalar(xbn[:, 1, :, 1:129], xsb[:, 1, :, :],
                                 st1[:, 19:20], st1[:, 21:22],
                                 op0=OP.mult, op1=OP.add)
                edge_mask(xbn[:, 0, 1, :], 0)
                edge_mask(xbn[:, 1, 1, :], 0)
                edge_mask(xbn[:, 0, 34, :], 1)
                edge_mask(xbn[:, 1, 34, :], 1)

            # ---- fused half-block: conv2(gw) -> depthwise -> 1x1 -> gate ----
            LAG = 1

            def gw_matmuls(ps, npx, gwsb, gwf, pairs, f16taps, dqsrc, hsrc,
                           r0, nr, g):
                # DoubleRow fp8 passes (paired taps on the mean-removed dq)
                # then plain fp16 passes (leftover taps on zero-padded h).
                for pi, (pa, pb) in enumerate(pairs):
                    sy0, sx0 = divmod(pa, 3)
                    sy1, sx1 = divmod(pb, 3)
                    a = dqsrc[:, r0 + sy0: r0 + sy0 + nr, sx0:sx0 + 128]
                    delta = (sy1 - sy0) * 130 + (sx1 - sx0)
                    rhs = APc(a.tensor, a.offset,
                              [list(a.ap[0]), [delta, 2], [130, nr], [1, 128]])
                    te.matmul(ps[:, :npx], gwsb[:, pa:pa + 2, g, :], rhs,
                              start=(pi == 0), stop=False, perf_mode=DR,
                              skip_group_check=True)
                for fi, t in enumerate(f16taps):
                    sy, sx = divmod(t, 3)
                    te.matmul(ps[:, :npx], gwf[:, fi, g, :],
                              hsrc[:, r0 + sy: r0 + sy + nr, sx:sx + 128],
                              start=False, stop=(fi == len(f16taps) - 1),
                              skip_group_check=True)

            def halfblock(first, gwsb, gwf, pairs, f16taps, dqsrc, hsrc, xsrc,
                          btbuf, csb, cbc, swsrc, swrows,
                          do_dc=True, do_flush=True, defer_tail=False,
                          on_flush=None, after_tile0=None):
                tiles = _row_tiles(ROWS_B if first else ROWS_C)
                if not first:
                    # split the final tile so the end-of-kernel pipeline drain
                    # (depthconv + 1x1 + gate + residual) is half as deep
                    (r0l, _) = tiles[-1]
                    tiles = tiles[:-1] + [(r0l, 2), (r0l + 2, 2)]
                pend = []

                def flush_one(item):
                    idx, r0, nr, dps = item
                    npx = nr * 128
                    if not do_flush:
                        return
                    for hh in range(2):
                        up = pu.tile([128, 512], F32, tag="pu",
                                     name=f"up{int(first)}_{idx}_{hh}")
                        te.matmul(up[:, :npx], csb[:, 0, hh, :], dps[0][:, :nr, :],
                                  start=True, stop=False)
                        te.matmul(up[:, :npx], csb[:, 1, hh, :], dps[1][:, :nr, :],
                                  start=False, stop=True)
                        z = workp.tile([128, 512], F16, tag="z", bufs=1,
                                       name=f"z{int(first)}_{idx}_{hh}")
                        ve.scalar_tensor_tensor(
                            z[:, :npx], up[:, :npx], bv[:, cbc + hh:cbc + hh + 1],
                            swsrc[:, hh, r0:r0 + nr, :],
                            op0=OP.add, op1=OP.mult)
                        if first:
                            ve.scalar_tensor_tensor(
                                dxm[:, hh, r0:r0 + nr, 1:129], z[:, :npx], 0.2,
                                z[:, :npx], op0=OP.mult, op1=OP.max)
                        else:
                            d2 = workp.tile([128, 512], F16, tag="d2", bufs=1,
                                            name=f"d2_{idx}_{hh}")
                            ve.scalar_tensor_tensor(
                                d2[:, :npx], z[:, :npx], 0.2, z[:, :npx],
                                op0=OP.mult, op1=OP.max)
                            # residual rows re-fetched from the fp16 x input
                            # (xsb's SBUF slot is reused by bn2dx by now)
                            xrt = workp.tile([128, 512], F16, tag="xr",
                                             name=f"xr_{idx}_{hh}")
                            sync.dma_start(xrt[:, :npx],
                                           x_d.ap()[:, hh, r0 + 2:r0 + 2 + nr, :])
                            stg = workp.tile([128, 512], F32, tag="st",
                                             name=f"st_{idx}_{hh}")
                            ve.tensor_add(stg[:, :npx], d2[:, :npx], xrt[:, :npx])
                            sync.dma_start(out_d.ap()[:, hh, r0:r0 + nr, :], stg[:, :npx])
                    if on_flush is not None:
                        on_flush(idx, r0, nr)

                for idx, (r0, nr) in enumerate(tiles):
                    npx = nr * 128
                    cwt = cwtp.tile([128, 18, 512], F16, tag="cwt",
                                    name=f"cwt{int(first)}_{idx}")
                    for g in range(18):
                        ps = pcw.tile([128, 512], F32, tag="pcw",
                                      name=f"pcw{int(first)}_{idx}_{g}")
                        gw_matmuls(ps, npx, gwsb, gwf, pairs, f16taps,
                                   dqsrc, hsrc, r0, nr, g)
                        act.activation(cwt[:, g, :npx], ps[:, :npx],
                                       AF.Identity, scale=1.0 / WSC,
                                       bias=btbuf[:, g:g + 1])
                    if idx == 0 and after_tile0 is not None:
                        # AR-independent PE work emitted before the first
                        # flush: covers the BN2-AllReduce wait at the B->C
                        # boundary (the in-order PE would otherwise stall at
                        # the first 1x1, which depends on bn2dx)
                        after_tile0()
                    dps = []
                    for hh in range(2 if do_dc else 0):
                        dp = workp.tile([128, 4, 128], F16, tag=f"dp{hh}",
                                        name=f"dp{int(first)}_{idx}_{hh}")
                        tmp = workp.tile([128, 512], F16, tag="tmp", bufs=1,
                                         name=f"tmp{int(first)}_{idx}_{hh}")
                        for t in range(9):
                            ty, tx = divmod(t, 3)
                            src = xsrc[:, hh, r0 + ty: r0 + ty + nr, tx:tx + 128]
                            if t == 0:
                                ve.tensor_mul(dp[:, :nr, :], cwt[:, hh, :npx], src)
                            else:
                                ve.tensor_mul(tmp[:, :npx], cwt[:, t * 2 + hh, :npx], src)
                                ve.tensor_add(dp[:, :nr, :], dp[:, :nr, :], tmp[:, :npx])
                        dps.append(dp)
                    pend.append((idx, r0, nr, dps))
                    if len(pend) > LAG:
                        flush_one(pend.pop(0))
                if defer_tail:
                    return pend, flush_one
                while pend:
                    flush_one(pend.pop(0))
                return None

            if level < 6:
                bstg = static.tile([128, 128], F32, tag="bstg", name="bstg")
                ve.memset(bstg[:], 0.0)
                sync.dma_start(out_d.ap()[:, 0, 0, :], bstg[:])

            # incremental BN2 stats: accumulate partial sums per flushed dxm
            # chunk so the AllReduce can start right after the last flush
            # instead of after four full-height stats passes.
            stp = static.tile([128, 4, 3], F32, tag="stpc", name="stpsb")
            BN2_CHUNKS = {2: (0, 1, 12), 5: (1, 12, 24), 8: (2, 24, 33)}

            def bn2_inc(idx, r0, nr):
                if level < 3.8 or idx not in BN2_CHUNKS:
                    return
                ci, lo, hi = BN2_CHUNKS[idx]
                for hh in range(2):
                    scr = cwtp.tile([128, 12, 130], F16, tag="cwt",
                                    name=f"scrB{ci}_{hh}")
                    nrw = hi - lo
                    act.activation(scr[:, :nrw, :], dxm[:, hh, lo:hi, :],
                                   AF.Identity, accum_out=stp[:, hh, ci:ci + 1])
                    act.activation(scr[:, :nrw, :], dxm[:, hh, lo:hi, :],
                                   AF.Square, accum_out=stp[:, 2 + hh, ci:ci + 1])

            tailB = None
            if level >= 3.2:
                tailB = halfblock(True, gw1, gw1f, GW1_PAIRS, GW1_F16, dq1, h1,
                                  xbn, bt1, c1, C1B, sw1, ROWS_B,
                                  do_dc=(level >= 3.4), do_flush=(level >= 3.6),
                                  defer_tail=(level >= 6),
                                  on_flush=bn2_inc if level >= 3.6 else None)

            if level >= 5:
                # Emit phase-C fmap convs before draining B's tail flushes so
                # the PE has independent work while the DVE finishes phase B.
                gw2 = static.tile([128, 8, 18, 128], F8, tag="gwA", name="gw2sb")
                sync.dma_start(gw2[:], gw2_d.ap())
                gw2f = static.tile([128, 1, 18, 128], F16, tag="gwF", name="gw2fsb")
                sync.dma_start(gw2f[:], gw2f_d.ap())
                se2 = static.tile([128, 9, 2, 128], F8, tag="seC", name="se2sb")
                sync.dma_start(se2[:], se2_d.ap())
                h2se = static.tile([128, 34, 130], F16, tag="hB", name="h2sesb")
                dq2se = static.tile([128, 34, 130], F8, tag="dqS", name="dq2sesb")
                sw2 = static.tile([128, 2, 32, 128], F16, tag="swc", name="sw2sb")
                bn2dx = static.tile([128, 2, 34, 130], F16, tag="wbig2",
                                    name="bn2dxsb")
                zero_pads(h2se)
                zero_pads(bn2dx)
                hconv(h2se, 3, 1, 34, (0, 33))
                mean_stats(h2se, dq2se, 3, 34)
                mean_bias(3, wse, 1, bt2se, SB2, 2)
                seconv(sw2, dq2se, se2, bt2se, 32)

            if tailB is not None:
                pendB, flushB = tailB
                while pendB:
                    flushB(pendB.pop(0))

            if level >= 3.8:
                # reduce the 3 per-chunk partial sums into st2[:, 0:4]
                # (layout stp[128, stat j, chunk]: j = {sum_h0,sum_h1,sq_h0,sq_h1})
                ve.tensor_add(st2[:, 0:4], stp[:, :, 0], stp[:, :, 1])
                ve.tensor_add(st2[:, 0:4], st2[:, 0:4], stp[:, :, 2])
            if level >= 4:
                gp.dma_start(cc2i[:], st2[:, 0:4])
                gp.collective_compute(
                    "AllReduce", OP.add, replica_groups=[list(range(N_CORES))],
                    ins=[cc2i.opt()], outs=[cc2o.opt()])
                gp.dma_start(st2[:, 4:8], cc2o[:])

            if level >= 5:
                bn_coeffs(st2, BN2G, BN2B)
                # one half ScalarE, one DVE: halves the post-AllReduce latency
                act.activation(bn2dx[:, 0, :, 1:129], dxm[:, 0, :, 1:129],
                               AF.Identity, scale=st2[:, 18:19],
                               bias=st2[:, 20:21])
                ve.tensor_scalar(bn2dx[:, 1, :, 1:129], dxm[:, 1, :, 1:129],
                                 st2[:, 19:20], st2[:, 21:22],
                                 op0=OP.mult, op1=OP.add)
                edge_mask(bn2dx[:, 0, 0, :], 0)
                edge_mask(bn2dx[:, 1, 0, :], 0)
                edge_mask(bn2dx[:, 0, 33, :], 1)
                edge_mask(bn2dx[:, 1, 33, :], 1)

            if level >= 6:
                halfblock(False, gw2, gw2f, GW2_PAIRS, GW2_F16, dq2, h2,
                          bn2dx, bt2, c3, C3B, sw2, ROWS_C)

    nc.compile()
    return nc


# ---------------------------------------------------------------------------
# Host-side sharding / layout prep
# ---------------------------------------------------------------------------

def _prep_weights(inp):
    """Shared (shard-independent) weight/bias layout prep."""
    def f16a(a):
        return np.ascontiguousarray(a.astype(F16NP))

    out = {}
    # first convs, K=105 (sy*35+cin), per sx, per conv j in (gw1, gw2, se1, se2)
    w1l = np.zeros((105, 3, 4, 128), np.float32)
    for j, wkey in enumerate(("gw1_w1", "gw2_w1", "se1_w1", "se2_w1")):
        wj = inp[wkey]  # (128, 35, 3, 3)
        for sy in range(3):
            for sx in range(3):
                w1l[sy * 35:(sy + 1) * 35, sx, j, :] = wj[:, :, sy, sx].T
    out["w1l"] = f16a(w1l)

    m = np.arange(128)
    wsum = np.zeros((128, 2, 18, 128), np.float32)
    n8 = {0: 6, 1: 8}  # fp8 tap count per conv (rest go to the fp16 tensor)
    for jj, (key, fkey, src) in enumerate((("gw1l", "gw1f", "gw1_w2"),
                                           ("gw2l", "gw2f", "gw2_w2"))):
        wsrc = inp[src]  # (2304, 128, 3, 3)
        gl = np.zeros((128, 9, 18, 128), np.float32)
        for s9 in range(9):
            sy, sx = divmod(s9, 3)
            for g in range(18):
                t, hh = g // 2, g % 2
                rows = (hh * 128 + m) * 9 + t
                gl[:, s9, g, :] = wsrc[rows, :, sy, sx].T
        k = n8[jj]
        # c-trick bias uses the exact fp32 tap-sum of the fp8 taps only
        wsum[:, jj, :, :] = gl[:, :k].sum(axis=1)
        out[key] = np.ascontiguousarray((gl[:, :k] * WSC).astype(F8NP))
        # fp16 leftover taps pre-scaled by 32 (power of two: exact in fp16)
        # so the uniform 1/32 drain scale applies to the whole PSUM
        out[fkey] = np.ascontiguousarray((gl[:, k:] * WSC).astype(F16NP))
    out["wsuml"] = f16a(wsum)

    wse = np.zeros((128, 2, 2, 128), np.float32)
    for jj, (key, src) in enumerate((("se1l", "se1_w2"), ("se2l", "se2_w2"))):
        wsrc = inp[src]  # (256, 128, 3, 3)
        sl = np.zeros((128, 9, 2, 128), np.float32)
        for s9 in range(9):
            sy, sx = divmod(s9, 3)
            for hh in range(2):
                sl[:, s9, hh, :] = wsrc[hh * 128 + m, :, sy, sx].T
        wse[:, jj, :, :] = sl.sum(axis=1)
        out[key] = np.ascontiguousarray((sl * WSC).astype(F8NP))
    out["wssel"] = f16a(wse)

    for key, src in (("c1l", "conv1_w"), ("c3l", "conv3_w")):
        wsrc = inp[src][:, :, 0, 0]  # (256, 256) [cout, cin]
        cl = np.zeros((128, 2, 2, 128), np.float32)
        for kg in range(2):
            for hh in range(2):
                # cl[k, kg, hh, mo] = w[hh*128+mo, kg*128+k]
                cl[:, kg, hh, :] = wsrc[hh * 128:(hh + 1) * 128,
                                        kg * 128:(kg + 1) * 128].T
        out[key] = f16a(cl)
    return out


def _prep_bvec(inp, top_edge, bot_edge):
    bvec = np.zeros((128, 58), np.float32)
    for j, k in enumerate(("gw1_b1", "gw2_b1", "se1_b1", "se2_b1")):
        bvec[:, j] = inp[k]
    for base, k in ((4, "gw1_b2"), (22, "gw2_b2")):
        b2 = inp[k]
        for g in range(18):
            t, hh = g // 2, g % 2
            bvec[:, base + g] = b2[(hh * 128 + np.arange(128)) * 9 + t]
    for base, k in ((40, "se1_b2"), (42, "se2_b2"), (44, "conv1_b"), (46, "conv3_b"),
                    (48, "bn1_g"), (50, "bn1_b"), (52, "bn2_g"), (54, "bn2_b")):
        v = inp[k]
        bvec[:, base] = v[:128]
        bvec[:, base + 1] = v[128:]
    bvec[:, 56] = 0.0 if top_edge else 1.0
    bvec[:, 57] = 0.0 if bot_edge else 1.0
    return bvec


def _prep_shard(inp, i, wshared):
    n, blk = i // 4, i % 4
    s = RB * blk
    e = s + RB
    x = inp["x"][n]                       # (256,128,128)
    seg_ds = inp["seg"][n][:, ::2, ::2]   # (35,128,128)

    xg = x.reshape(2, 128, H, W)

    x_bf = np.zeros((128, 2, 36, 128), F16NP)
    lo, hi = max(s - 2, 0), min(e + 2, H)
    x_bf[:, :, lo - (s - 2):hi - (s - 2), :] = \
        xg[:, :, lo:hi, :].transpose(1, 0, 2, 3).astype(F16NP)

    seg3 = np.zeros((105, 36, 130), F16NP)
    seg_f = seg_ds.astype(F16NP)
    for sy in range(3):
        # seg3[sy*35+c, j, 1+x] = segmap[c, s-3+j+sy, x]
        r0 = s - 3 + sy
        lo, hi = max(r0, 0), min(r0 + 36, H)
        if hi > lo:
            seg3[sy * 35:(sy + 1) * 35, lo - r0:hi - r0, 1:129] = seg_f[:, lo:hi, :]

    return {
        "x_bf": x_bf,
        "seg3": np.ascontiguousarray(seg3),
        "bvec": _prep_bvec(inp, s == 0, e == H),
        **wshared,
    }


def kernel(**inputs):
    inp = {k: np.asarray(v) for k, v in inputs.items()}

    if "nc" not in _CACHE:
        _CACHE["nc"] = _build()
    nc = _CACHE["nc"]

    wshared = _prep_weights(inp)
    in_maps = [_prep_shard(inp, i, wshared) for i in range(N_CORES)]

    from concourse.bass_utils import run_bass_kernel_spmd
    res = run_bass_kernel_spmd(nc, in_maps, core_ids=list(range(N_CORES)),
                               trace=False)

    out = np.zeros((N, C, H, W), np.float32)
    for i in range(N_CORES):
        n, blk = i // 4, i % 4
        s = RB * blk
        o = res.results[i]["out"]  # (128, 2, 32, 128)
        out[n, :, s:s + RB, :] = o.transpose(1, 0, 2, 3).reshape(C, RB, W)
    return out


# revision 57
# speedup vs baseline: 1.0423x; 1.0013x over previous
"""Trainium2 Bass kernel for nn_DepthsepCCBlock (dense_cnn).

Strategy: 8-way shard over (batch=2) x (H/4 blocks of 32 rows). Each core
computes its 32 output rows end-to-end. The two training-mode BatchNorms
become sync-BN via two tiny (128x4 fp32) AllReduces, both fully overlapped
with TensorEngine work. The dominant 128->2304 3x3 convs run in fp8 e4m3
DoubleRow mode (two taps contracted per PE pass: 9 taps in 5 passes, 1.8x).
Precision is preserved by mean-removal: the conv input is delta = h - c
(c = per-channel mean, pads become -c automatically since h pads are 0) and
the exact-fp32 tap-sum-of-weights times c is folded back in via a per-group
bias computed on device (18 tiny matmuls). This is cell-wise exact for any
c, so only the fp8 quantization of delta and of the weights remains as
error. All other tensors run in fp16 (same PE/DVE speed as bf16, 8x less
noise). The per-pixel dynamic depthwise 3x3 conv runs on the VectorEngine
as 17 tensor-tensor ops per tile, fused with the producing conv. Halo rows
are recomputed from host-sliced zero-padded shards (no halo exchange);
image-boundary BN-bias artifacts are killed with per-core edge-row masks
supplied as data so every core runs one identical NEFF.
"""
import sys
import numpy as np
import ml_dtypes

if "/opt/trn_rl_repo" not in sys.path:
    sys.path.insert(0, "/opt/trn_rl_repo")

F16NP = np.float16
F8NP = ml_dtypes.float8_e4m3

N, C, H, W = 2, 256, 128, 128
SNC, NH = 35, 128
EPS = 1e-5
N_CORES = 8
RB = 32                      # output rows per shard
CNT = float(N * H * W)       # BN reduction count per channel
WSC = 32.0                   # fp8 weight scale

ROWS_B = 34                  # dx_mid rows (s-1 .. e)
ROWS_C = 32                  # output rows (s .. e-1)

# fp8 tap allocation per gw conv: gw1 taps 0-5 via 3 DoubleRow pairs (taps
# 6,7,8 stay fp16), gw2 taps 0-7 via 4 pairs (tap 8 fp16). Chosen from the
# precision sim: (6,8) taps fp8 -> rel err ~1.6e-2 vs 1.84e-2 for (9,9),
# at the cost of one extra PE pass on gw1 only.
GW1_PAIRS = [(0, 1), (2, 3), (4, 5)]
GW1_F16 = [6, 7, 8]
GW2_PAIRS = [(0, 1), (2, 3), (4, 5), (6, 7)]
GW2_F16 = [8]


def _row_tiles(nrows):
    out = []
    r = 0
    while r < nrows:
        nr = min(4, nrows - r)
        out.append((r, nr))
        r += nr
    return out


_CACHE = {}


def _build(level=6):
    from contextlib import ExitStack
    import concourse.tile as tile
    from concourse import bacc, mybir
    from concourse.ap import AP as APc

    F32 = mybir.dt.float32
    F16 = mybir.dt.float16
    F8 = mybir.dt.float8e4
    AF = mybir.ActivationFunctionType
    OP = mybir.AluOpType
    DR = mybir.MatmulPerfMode.DoubleRow

    nc = bacc.Bacc("TRN2", target_bir_lowering=False, debug=False,
                   num_devices=N_CORES)

    x_d = nc.dram_tensor("x_bf", [128, 2, 36, 128], F16, kind="ExternalInput")
    seg_d = nc.dram_tensor("seg3", [105, 36, 130], F16, kind="ExternalInput")
    w1_d = nc.dram_tensor("w1l", [105, 3, 4, 128], F16, kind="ExternalInput")
    gw1_d = nc.dram_tensor("gw1l", [128, 6, 18, 128], F8, kind="ExternalInput")
    gw2_d = nc.dram_tensor("gw2l", [128, 8, 18, 128], F8, kind="ExternalInput")
    gw1f_d = nc.dram_tensor("gw1f", [128, 3, 18, 128], F16, kind="ExternalInput")
    gw2f_d = nc.dram_tensor("gw2f", [128, 1, 18, 128], F16, kind="ExternalInput")
    ws_d = nc.dram_tensor("wsuml", [128, 2, 18, 128], F16, kind="ExternalInput")
    wse_d = nc.dram_tensor("wssel", [128, 2, 2, 128], F16, kind="ExternalInput")
    se1_d = nc.dram_tensor("se1l", [128, 9, 2, 128], F8, kind="ExternalInput")
    se2_d = nc.dram_tensor("se2l", [128, 9, 2, 128], F8, kind="ExternalInput")
    c1_d = nc.dram_tensor("c1l", [128, 2, 2, 128], F16, kind="ExternalInput")
    c3_d = nc.dram_tensor("c3l", [128, 2, 2, 128], F16, kind="ExternalInput")
    bv_d = nc.dram_tensor("bvec", [128, 58], F32, kind="ExternalInput")
    out_d = nc.dram_tensor("out", [128, 2, 32, 128], F32, kind="ExternalOutput")

    # bvec columns
    B1C = 0          # 4: first-conv biases (h1, h2, h1se, h2se order: gw1,gw2,se1,se2)
    GB1 = 4          # 18: gw1_b2 per (t,half)
    GB2 = 22         # 18: gw2_b2
    SB1 = 40         # 2: se1_b2 ; SB2=42 ; C1B=44 ; C3B=46
    SB2, C1B, C3B = 42, 44, 46
    BN1G, BN1B, BN2G, BN2B = 48, 50, 52, 54
    EMSK = 56        # 2: top,bot edge masks

    with tile.TileContext(nc) as tc:
        with ExitStack() as ctx:
            static = ctx.enter_context(tc.tile_pool(name="static", bufs=1))
            cwtp = ctx.enter_context(tc.tile_pool(name="cwtp", bufs=2))
            workp = ctx.enter_context(tc.tile_pool(name="work", bufs=2))
            dramp = ctx.enter_context(tc.tile_pool(name="dramp", bufs=1, space="DRAM"))
            ph = ctx.enter_context(tc.tile_pool(name="ph", bufs=2, space="PSUM"))
            pcw = ctx.enter_context(tc.tile_pool(name="pcw", bufs=4, space="PSUM"))
            pu = ctx.enter_context(tc.tile_pool(name="pu", bufs=2, space="PSUM"))

            # ---- static SBUF tiles ----
            xsb = static.tile([128, 2, 36, 128], F16, tag="wbig2", name="xsb")
            seg = static.tile([105, 36, 130], F16, tag="segc", name="segsb")
            w1 = static.tile([105, 3, 4, 128], F16, tag="w1c", name="w1sb")
            gw1 = static.tile([128, 6, 18, 128], F8, tag="gwA", name="gw1sb")
            gw1f = static.tile([128, 3, 18, 128], F16, tag="gwF", name="gw1fsb")
            ws = static.tile([128, 2, 18, 128], F16, tag="wsc", name="wssb")
            wse = static.tile([128, 2, 2, 128], F16, tag="wsec", name="wsesb")
            se1 = static.tile([128, 9, 2, 128], F8, tag="seC", name="se1sb")
            dq1se = static.tile([128, 36, 130], F8, tag="dqS", name="dq1sesb")
            bt1se = static.tile([128, 2], F32, tag="btsec", name="bt1sesb")
            bt2se = static.tile([128, 2], F32, tag="btse2c", name="bt2sesb")
            c1 = static.tile([128, 2, 2, 128], F16, tag="c1c", name="c1sb")
            c3 = static.tile([128, 2, 2, 128], F16, tag="c3c", name="c3sb")
            bv = static.tile([128, 58], F32, tag="bvc", name="bvsb")
            h1 = static.tile([128, 36, 130], F16, tag="hB", name="h1sb")
            h1se = static.tile([128, 36, 130], F16, tag="hA", name="h1sesb")
            dq1 = static.tile([128, 36, 130], F8, tag="dqA", name="dq1sb")
            xbn = static.tile([128, 2, 36, 130], F16, tag="xbnc", name="xbnsb")
            sw1 = static.tile([128, 2, 34, 128], F16, tag="swc", name="sw1sb")
            dxm = static.tile([128, 2, 34, 130], F16, tag="dxmc", name="dxmsb")
            st1 = static.tile([128, 22], F32, tag="st1c", name="st1sb")
            st2 = static.tile([128, 22], F32, tag="st2c", name="st2sb")
            cv = static.tile([128, 4], F16, tag="cvc", name="cvsb")
            cv32 = static.tile([128, 4], F32, tag="cv32c", name="cv32sb")
            hsum = static.tile([128, 4, 9], F32, tag="hsumc", name="hsumsb")
            bt1 = static.tile([128, 18], F32, tag="bt1c", name="bt1sb")
            bt2 = static.tile([128, 18], F32, tag="bt2c", name="bt2sb")

            cc1i = dramp.tile([128, 4], F32, name="cc1i")
            cc1o = dramp.tile([128, 4], F32, addr_space="Shared", name="cc1o")
            cc2i = dramp.tile([128, 4], F32, name="cc2i")
            cc2o = dramp.tile([128, 4], F32, addr_space="Shared", name="cc2o")

            sync, ve, act, gp, te = nc.sync, nc.vector, nc.scalar, nc.gpsimd, nc.tensor

            # ---- input DMAs. One queue, critical-first: the first conv needs
            # only seg/w1/bv, and a single in-order queue guarantees the big
            # loads don't steal HBM bandwidth from them (a multi-queue spread
            # measured 23us slower to first matmul). The first 12 seg rows
            # (3 conv tiles) ship ahead of the bulk so the PE starts sooner.
            sync.dma_start(seg[:, 0:12, :], seg_d.ap()[:, 0:12, :])
            sync.dma_start(w1[:], w1_d.ap())
            sync.dma_start(bv[:], bv_d.ap())
            sync.dma_start(seg[:, 12:36, :], seg_d.ap()[:, 12:36, :])
            sync.dma_start(xsb[:], x_d.ap())
            sync.dma_start(se1[:], se1_d.ap())
            sync.dma_start(ws[:], ws_d.ap())
            sync.dma_start(wse[:], wse_d.ap())
            sync.dma_start(gw1[:], gw1_d.ap())
            sync.dma_start(gw1f[:], gw1f_d.ap())
            sync.dma_start(c1[:], c1_d.ap())
            sync.dma_start(c3[:], c3_d.ap())

            # ---- zero pad cells. Every row and all interior columns of these
            # buffers get written by drains/affines before any read, so only
            # the two pad columns need zeroing.
            def zero_pads(buf):
                if len(buf.shape) == 4:
                    ve.memset(buf[:, :, :, 0], 0.0)
                    ve.memset(buf[:, :, :, 129], 0.0)
                else:
                    ve.memset(buf[:, :, 0], 0.0)
                    ve.memset(buf[:, :, 129], 0.0)

            for buf in (h1, h1se, xbn, dxm):
                zero_pads(buf)

            def bn1_stats():
                # Emitted after the h convs: ScalarE is the drain engine for
                # the first-conv PSUMs, and 15us of stats passes up front
                # stalls the PE behind the 2-bank ph pool.
                scrA = cwtp.tile([128, 32, 128], F16, tag="cwt", name="scrA")
                for hh in range(2):
                    act.activation(scrA[:], xsb[:, hh, 2:34, :], AF.Identity,
                                   accum_out=st1[:, hh:hh + 1])
                    act.activation(scrA[:], xsb[:, hh, 2:34, :], AF.Square,
                                   accum_out=st1[:, 2 + hh:3 + hh])
                # cc bounce DMAs ride the otherwise-idle gpsimd queue so they
                # are not serialized behind the weight loads on sync's queue.
                gp.dma_start(cc1i[:], st1[:, 0:4])
                gp.collective_compute(
                    "AllReduce", OP.add, replica_groups=[list(range(N_CORES))],
                    ins=[cc1i.opt()], outs=[cc1o.opt()])
                gp.dma_start(st1[:, 4:8], cc1o[:])

            def edge_mask(buf_row_ap, mcol):
                ve.tensor_scalar(buf_row_ap, buf_row_ap,
                                 bv[:, EMSK + mcol:EMSK + mcol + 1], None,
                                 op0=OP.mult)

            # ---- first convs: h = relu(conv(segmap)+b), K=105 (3 sy packed).
            # Each drain also accumulates its tile's h-sum (free on ScalarE):
            # feeds the mean-removal c with no extra stats pass. The sums see
            # pre-edge-mask values, which is fine — the c-trick is exact for
            # any c. ----
            def hconv(hbuf, cidx, jof, nrows, mrows):
                for ti, (r0, nr) in enumerate(_row_tiles(nrows)):
                    npx = nr * 128
                    ps = ph.tile([128, 512], F32, tag="ph", name=f"ps_h{cidx}_{r0}")
                    for sx in range(3):
                        te.matmul(ps[:, :npx], w1[:, sx, cidx, :],
                                  seg[:, jof + r0: jof + r0 + nr, sx:sx + 128],
                                  start=(sx == 0), stop=(sx == 2))
                    act.activation(hbuf[:, r0:r0 + nr, 1:129], ps[:, :npx],
                                   AF.Relu, bias=bv[:, B1C + cidx:B1C + cidx + 1],
                                   accum_out=hsum[:, cidx, ti:ti + 1])
                edge_mask(hbuf[:, mrows[0], :], 0)
                edge_mask(hbuf[:, mrows[1], :], 1)

            # ---- mean-removal: c = mean(h); dq = h - c (fp8, pads -> -c);
            # bias_g = (Wsum^T c)_g + b2_g via ng tiny matmuls ----
            def mean_stats(hbuf, dqbuf, ccol, nrows):
                # DVE-only: tree-reduce the per-tile drain sums into c, then
                # cast dq = h - c. No full-buffer scan needed.
                s = hsum[:, ccol, :]
                n = 9
                while n > 1:
                    hn = n // 2
                    ve.tensor_add(s[:, 0:hn], s[:, 0:hn], s[:, n - hn:n])
                    n = n - hn
                # c is materialized in fp16 (for the fp16 bias matvec) and
                # upcast to an exact fp32 twin for the DVE subtract, so both
                # consumers see bit-identical c and the correction is exact.
                ve.tensor_scalar_mul(cv[:, ccol:ccol + 1], s[:, 0:1],
                                     1.0 / (nrows * 130.0))
                act.activation(cv32[:, ccol:ccol + 1], cv[:, ccol:ccol + 1],
                               AF.Identity)
                ve.tensor_scalar(dqbuf[:, :nrows, :], hbuf[:, :nrows, :],
                                 cv32[:, ccol:ccol + 1], None, op0=OP.subtract)

            def mean_bias(ccol, wstile, wsj, btbuf, b2c0, ng):
                # PE matvec: emitted late enough that c is already computed,
                # so the in-order PE never blocks on the stats chain.
                # One accumulation group writing disjoint columns: start only
                # on g=0 (start marks the whole 2KB PSUM bank pending-zero;
                # later columns first-touch-zero their own bytes).
                psb = ph.tile([128, ng], F32, tag="ph", name=f"psb{ccol}")
                for g in range(ng):
                    te.matmul(psb[:, g:g + 1], wstile[:, wsj, g, :],
                              cv[:, ccol:ccol + 1], start=(g == 0),
                              stop=(g == ng - 1), skip_group_check=True)
                ve.tensor_add(btbuf[:], psb[:], bv[:, b2c0:b2c0 + ng])

            # ---- se conv: sw = sigmoid(conv3x3+b), fp8 DoubleRow on the
            # mean-removed dq (4 pairs + 1 single, scale-32 weights) ----
            def seconv(swbuf, dqsrc, sesb, btse, nrows):
                for (r0, nr) in _row_tiles(nrows):
                    npx = nr * 128
                    for hh in range(2):
                        ps = pcw.tile([128, 512], F32, tag="pcw",
                                      name=f"ps_se{nrows}_{r0}_{hh}")
                        for pi, (pa, pb) in enumerate(GW2_PAIRS):
                            sy0, sx0 = divmod(pa, 3)
                            sy1, sx1 = divmod(pb, 3)
                            a = dqsrc[:, r0 + sy0: r0 + sy0 + nr, sx0:sx0 + 128]
                            delta = (sy1 - sy0) * 130 + (sx1 - sx0)
                            rhs = APc(a.tensor, a.offset,
                                      [list(a.ap[0]), [delta, 2], [130, nr],
                                       [1, 128]])
                            te.matmul(ps[:, :npx], sesb[:, pa:pa + 2, hh, :], rhs,
                                      start=(pi == 0), stop=False, perf_mode=DR,
                                      skip_group_check=True)
                        te.matmul(ps[:, :npx], sesb[:, 8, hh, :],
                                  dqsrc[:, r0 + 2: r0 + 2 + nr, 2:2 + 128],
                                  start=False, stop=True, skip_group_check=True)
                        act.activation(swbuf[:, hh, r0:r0 + nr, :], ps[:, :npx],
                                       AF.Sigmoid, scale=1.0 / WSC,
                                       bias=btse[:, hh:hh + 1])

            h2 = static.tile([128, 34, 130], F16, tag="hA", name="h2sb")
            dq2 = static.tile([128, 34, 130], F8, tag="dqA", name="dq2sb")
            if level >= 2:
                hconv(h1, 0, 0, 36, (1, 34))
                # stats pass runs on ScalarE while the PE continues with
                # h1se's convs, so the bias matvec mostly doesn't stall
                if level >= 3.1:
                    mean_stats(h1, dq1, 0, 36)
                    mean_bias(0, ws, 0, bt1, GB1, 18)
                hconv(h1se, 2, 0, 36, (1, 34))
                if level >= 3.1:
                    mean_stats(h1se, dq1se, 2, 36)
                    mean_bias(2, wse, 0, bt1se, SB1, 2)
            bn1_stats()
            if level >= 3:
                seconv(sw1, dq1se, se1, bt1se, 34)

            if level >= 5:
                # h2 computed early: its slot (h1se's) frees after the sw1
                # conv, this PE work extends the window that hides the BN1
                # AllReduce, and it removes the h2 stall at the B->C boundary.
                zero_pads(h2)
                hconv(h2, 1, 1, 34, (0, 33))
                mean_stats(h2, dq2, 1, 34)
                mean_bias(1, ws, 1, bt2, GB2, 18)

            # ---- BN coefficient computation (tiny [128,2] ops) ----
            def bn_coeffs(st, gcol, bcol):
                ve.tensor_scalar_mul(st[:, 8:10], st[:, 4:6], 1.0 / CNT)     # mu
                ve.tensor_scalar_mul(st[:, 10:12], st[:, 6:8], 1.0 / CNT)    # q
                ve.tensor_mul(st[:, 12:14], st[:, 8:10], st[:, 8:10])        # mu^2
                ve.tensor_sub(st[:, 12:14], st[:, 10:12], st[:, 12:14])      # var
                ve.tensor_scalar_add(st[:, 12:14], st[:, 12:14], EPS)        # +eps
                act.activation(st[:, 14:16], st[:, 12:14], AF.Sqrt)
                ve.reciprocal(st[:, 16:18], st[:, 14:16])                    # rstd
                ve.tensor_mul(st[:, 18:20], bv[:, gcol:gcol + 2], st[:, 16:18])  # a
                ve.tensor_mul(st[:, 20:22], st[:, 8:10], st[:, 18:20])
                ve.tensor_sub(st[:, 20:22], bv[:, bcol:bcol + 2], st[:, 20:22])  # b
            # NOTE: bn stats sums land in st[:,4:8] post-allreduce; mean_remove
            # uses st1[:,8:9]/st2[:,8:9] as scratch for the h sums BEFORE
            # bn_coeffs overwrites st[:,8:10]; ordering below guarantees the
            # c values are consumed (cast+bias matmuls) before bn_coeffs runs.

            if level >= 3:
                bn_coeffs(st1, BN1G, BN1B)
                # xbn = a1*x + bb1 (interior cols), then edge-row masks.
                # One half on ScalarE, one on DVE to halve the latency.
                act.activation(xbn[:, 0, :, 1:129], xsb[:, 0, :, :],
                               AF.Identity, scale=st1[:, 18:19],
                               bias=st1[:, 20:21])
                ve.tensor_scalar(xbn[:, 1, :, 1:129], xsb[:, 1, :, :],
                                 st1[:, 19:20], st1[:, 21:22],
                                 op0=OP.mult, op1=OP.add)
                edge_mask(xbn[:, 0, 1, :], 0)
                edge_mask(xbn[:, 1, 1, :], 0)
                edge_mask(xbn[:, 0, 34, :], 1)
                edge_mask(xbn[:, 1, 34, :], 1)

            # ---- fused half-block: conv2(gw) -> depthwise -> 1x1 -> gate ----
            LAG = 1

            def gw_matmuls(ps, npx, gwsb, gwf, pairs, f16taps, dqsrc, hsrc,
                           r0, nr, g):
                # DoubleRow fp8 passes (paired taps on the mean-removed dq)
                # then plain fp16 passes (leftover taps on zero-padded h).
                for pi, (pa, pb) in enumerate(pairs):
                    sy0, sx0 = divmod(pa, 3)
                    sy1, sx1 = divmod(pb, 3)
                    a = dqsrc[:, r0 + sy0: r0 + sy0 + nr, sx0:sx0 + 128]
                    delta = (sy1 - sy0) * 130 + (sx1 - sx0)
                    rhs = APc(a.tensor, a.offset,
                              [list(a.ap[0]), [delta, 2], [130, nr], [1, 128]])
                    te.matmul(ps[:, :npx], gwsb[:, pa:pa + 2, g, :], rhs,
                              start=(pi == 0), stop=False, perf_mode=DR,
                              skip_group_check=True)
                for fi, t in enumerate(f16taps):
                    sy, sx = divmod(t, 3)
                    te.matmul(ps[:, :npx], gwf[:, fi, g, :],
                              hsrc[:, r0 + sy: r0 + sy + nr, sx:sx + 128],
                              start=False, stop=(fi == len(f16taps) - 1),
                              skip_group_check=True)

            def halfblock(first, gwsb, gwf, pairs, f16taps, dqsrc, hsrc, xsrc,
                          btbuf, csb, cbc, swsrc, swrows,
                          do_dc=True, do_flush=True, defer_tail=False,
                          on_flush=None, after_tile0=None):
                tiles = _row_tiles(ROWS_B if first else ROWS_C)
                if not first:
                    # split the final tile so the end-of-kernel pipeline drain
                    # (depthconv + 1x1 + gate + residual) is half as deep
                    (r0l, _) = tiles[-1]
                    tiles = tiles[:-1] + [(r0l, 2), (r0l + 2, 2)]
                pend = []

                def flush_one(item):
                    idx, r0, nr, dps = item
                    npx = nr * 128
                    if not do_flush:
                        return
                    for hh in range(2):
                        up = pu.tile([128, 512], F32, tag="pu",
                                     name=f"up{int(first)}_{idx}_{hh}")
                        te.matmul(up[:, :npx], csb[:, 0, hh, :], dps[0][:, :nr, :],
                                  start=True, stop=False)
                        te.matmul(up[:, :npx], csb[:, 1, hh, :], dps[1][:, :nr, :],
                                  start=False, stop=True)
                        z = workp.tile([128, 512], F16, tag="z", bufs=1,
                                       name=f"z{int(first)}_{idx}_{hh}")
                        ve.scalar_tensor_tensor(
                            z[:, :npx], up[:, :npx], bv[:, cbc + hh:cbc + hh + 1],
                            swsrc[:, hh, r0:r0 + nr, :],
                            op0=OP.add, op1=OP.mult)
                        if first:
                            ve.scalar_tensor_tensor(
                                dxm[:, hh, r0:r0 + nr, 1:129], z[:, :npx], 0.2,
                                z[:, :npx], op0=OP.mult, op1=OP.max)
                        else:
                            d2 = workp.tile([128, 512], F16, tag="d2", bufs=1,
                                            name=f"d2_{idx}_{hh}")
                            ve.scalar_tensor_tensor(
                                d2[:, :npx], z[:, :npx], 0.2, z[:, :npx],
                                op0=OP.mult, op1=OP.max)
                            # residual rows re-fetched from the fp16 x input
                            # (xsb's SBUF slot is reused by bn2dx by now)
                            xrt = workp.tile([128, 512], F16, tag="xr",
                                             name=f"xr_{idx}_{hh}")
                            sync.dma_start(xrt[:, :npx],
                                           x_d.ap()[:, hh, r0 + 2:r0 + 2 + nr, :])
                            stg = workp.tile([128, 512], F32, tag="st",
                                             name=f"st_{idx}_{hh}")
                            ve.tensor_add(stg[:, :npx], d2[:, :npx], xrt[:, :npx])
                            sync.dma_start(out_d.ap()[:, hh, r0:r0 + nr, :], stg[:, :npx])
                    if on_flush is not None:
                        on_flush(idx, r0, nr)

                for idx, (r0, nr) in enumerate(tiles):
                    npx = nr * 128
                    cwt = cwtp.tile([128, 18, 512], F16, tag="cwt",
                                    name=f"cwt{int(first)}_{idx}")
                    for g in range(18):
                        ps = pcw.tile([128, 512], F32, tag="pcw",
                                      name=f"pcw{int(first)}_{idx}_{g}")
                        gw_matmuls(ps, npx, gwsb, gwf, pairs, f16taps,
                                   dqsrc, hsrc, r0, nr, g)
                        act.activation(cwt[:, g, :npx], ps[:, :npx],
                                       AF.Identity, scale=1.0 / WSC,
                                       bias=btbuf[:, g:g + 1])
                    if idx == 0 and after_tile0 is not None:
                        # AR-independent PE work emitted before the first
                        # flush: covers the BN2-AllReduce wait at the B->C
                        # boundary (the in-order PE would otherwise stall at
                        # the first 1x1, which depends on bn2dx)
                        after_tile0()
                    dps = []
                    for hh in range(2 if do_dc else 0):
                        dp = workp.tile([128, 4, 128], F16, tag=f"dp{hh}",
                                        name=f"dp{int(first)}_{idx}_{hh}")
                        tmp = workp.tile([128, 512], F16, tag="tmp", bufs=1,
                                         name=f"tmp{int(first)}_{idx}_{hh}")
                        for t in range(9):
                            ty, tx = divmod(t, 3)
                            src = xsrc[:, hh, r0 + ty: r0 + ty + nr, tx:tx + 128]
                            if t == 0:
                                ve.tensor_mul(dp[:, :nr, :], cwt[:, hh, :npx], src)
                            else:
                                ve.tensor_mul(tmp[:, :npx], cwt[:, t * 2 + hh, :npx], src)
                                ve.tensor_add(dp[:, :nr, :], dp[:, :nr, :], tmp[:, :npx])
                        dps.append(dp)
                    pend.append((idx, r0, nr, dps))
                    if len(pend) > LAG:
                        flush_one(pend.pop(0))
                if defer_tail:
                    return pend, flush_one
                while pend:
                    flush_one(pend.pop(0))
                return None

            if level < 6:
                bstg = static.tile([128, 128], F32, tag="bstg", name="bstg")
                ve.memset(bstg[:], 0.0)
                sync.dma_start(out_d.ap()[:, 0, 0, :], bstg[:])

            # incremental BN2 stats: accumulate partial sums per flushed dxm
            # chunk so the AllReduce can start right after the last flush
            # instead of after four full-height stats passes.
            stp = static.tile([128, 4, 3], F32, tag="stpc", name="stpsb")
            BN2_CHUNKS = {2: (0, 1, 12), 5: (1, 12, 24), 8: (2, 24, 33)}

            def bn2_inc(idx, r0, nr):
                if level < 3.8 or idx not in BN2_CHUNKS:
                    return
                ci, lo, hi = BN2_CHUNKS[idx]
                for hh in range(2):
                    scr = cwtp.tile([128, 12, 130], F16, tag="cwt",
                                    name=f"scrB{ci}_{hh}")
                    nrw = hi - lo
                    act.activation(scr[:, :nrw, :], dxm[:, hh, lo:hi, :],
                                   AF.Identity, accum_out=stp[:, hh, ci:ci + 1])
                    act.activation(scr[:, :nrw, :], dxm[:, hh, lo:hi, :],
                                   AF.Square, accum_out=stp[:, 2 + hh, ci:ci + 1])

            tailB = None
            if level >= 3.2:
                tailB = halfblock(True, gw1, gw1f, GW1_PAIRS, GW1_F16, dq1, h1,
                                  xbn, bt1, c1, C1B, sw1, ROWS_B,
                                  do_dc=(level >= 3.4), do_flush=(level >= 3.6),
                                  defer_tail=(level >= 6),
                                  on_flush=bn2_inc if level >= 3.6 else None)

            if level >= 5:
                # Emit phase-C fmap convs before draining B's tail flushes so
                # the PE has independent work while the DVE finishes phase B.
                gw2 = static.tile([128, 8, 18, 128], F8, tag="gwA", name="gw2sb")
                sync.dma_start(gw2[:], gw2_d.ap())
                gw2f = static.tile([128, 1, 18, 128], F16, tag="gwF", name="gw2fsb")
                sync.dma_start(gw2f[:], gw2f_d.ap())
                se2 = static.tile([128, 9, 2, 128], F8, tag="seC", name="se2sb")
                sync.dma_start(se2[:], se2_d.ap())
                h2se = static.tile([128, 34, 130], F16, tag="hB", name="h2sesb")
                dq2se = static.tile([128, 34, 130], F8, tag="dqS", name="dq2sesb")
                sw2 = static.tile([128, 2, 32, 128], F16, tag="swc", name="sw2sb")
                bn2dx = static.tile([128, 2, 34, 130], F16, tag="wbig2",
                                    name="bn2dxsb")
                zero_pads(h2se)
                zero_pads(bn2dx)
                hconv(h2se, 3, 1, 34, (0, 33))
                mean_stats(h2se, dq2se, 3, 34)
                mean_bias(3, wse, 1, bt2se, SB2, 2)

            if tailB is not None:
                pendB, flushB = tailB
                while pendB:
                    flushB(pendB.pop(0))

            if level >= 3.8:
                # reduce the 3 per-chunk partial sums into st2[:, 0:4]
                # (layout stp[128, stat j, chunk]: j = {sum_h0,sum_h1,sq_h0,sq_h1})
                ve.tensor_add(st2[:, 0:4], stp[:, :, 0], stp[:, :, 1])
                ve.tensor_add(st2[:, 0:4], st2[:, 0:4], stp[:, :, 2])
            if level >= 4:
                gp.dma_start(cc2i[:], st2[:, 0:4])
                gp.collective_compute(
                    "AllReduce", OP.add, replica_groups=[list(range(N_CORES))],
                    ins=[cc2i.opt()], outs=[cc2o.opt()])
                gp.dma_start(st2[:, 4:8], cc2o[:])

            if level >= 5:
                bn_coeffs(st2, BN2G, BN2B)
                # one half ScalarE, one DVE: halves the post-AllReduce latency
                act.activation(bn2dx[:, 0, :, 1:129], dxm[:, 0, :, 1:129],
                               AF.Identity, scale=st2[:, 18:19],
                               bias=st2[:, 20:21])
                ve.tensor_scalar(bn2dx[:, 1, :, 1:129], dxm[:, 1, :, 1:129],
                                 st2[:, 19:20], st2[:, 21:22],
                                 op0=OP.mult, op1=OP.add)
                edge_mask(bn2dx[:, 0, 0, :], 0)
                edge_mask(bn2dx[:, 1, 0, :], 0)
                edge_mask(bn2dx[:, 0, 33, :], 1)
                edge_mask(bn2dx[:, 1, 33, :], 1)

            if level >= 6:
                halfblock(False, gw2, gw2f, GW2_PAIRS, GW2_F16, dq2, h2,
                          bn2dx, bt2, c3, C3B, sw2, ROWS_C,
                          after_tile0=lambda: seconv(sw2, dq2se, se2, bt2se, 32))

    nc.compile()
    return nc


# ---------------------------------------------------------------------------
# Host-side sharding / layout prep
# ---------------------------------------------------------------------------

def _prep_weights(inp):
    """Shared (shard-independent) weight/bias layout prep."""
    def f16a(a):
        return np.ascontiguousarray(a.astype(F16NP))

    out = {}
    # first convs, K=105 (sy*35+cin), per sx, per conv j in (gw1, gw2, se1, se2)
    w1l = np.zeros((105, 3, 4, 128), np.float32)
    for j, wkey in enumerate(("gw1_w1", "gw2_w1", "se1_w1", "se2_w1")):
        wj = inp[wkey]  # (128, 35, 3, 3)
        for sy in range(3):
            for sx in range(3):
                w1l[sy * 35:(sy + 1) * 35, sx, j, :] = wj[:, :, sy, sx].T
    out["w1l"] = f16a(w1l)

    m = np.arange(128)
    wsum = np.zeros((128, 2, 18, 128), np.float32)
    n8 = {0: 6, 1: 8}  # fp8 tap count per conv (rest go to the fp16 tensor)
    for jj, (key, fkey, src) in enumerate((("gw1l", "gw1f", "gw1_w2"),
                                           ("gw2l", "gw2f", "gw2_w2"))):
        wsrc = inp[src]  # (2304, 128, 3, 3)
        gl = np.zeros((128, 9, 18, 128), np.float32)
        for s9 in range(9):
            sy, sx = divmod(s9, 3)
            for g in range(18):
                t, hh = g // 2, g % 2
                rows = (hh * 128 + m) * 9 + t
                gl[:, s9, g, :] = wsrc[rows, :, sy, sx].T
        k = n8[jj]
        # c-trick bias uses the exact fp32 tap-sum of the fp8 taps only
        wsum[:, jj, :, :] = gl[:, :k].sum(axis=1)
        out[key] = np.ascontiguousarray((gl[:, :k] * WSC).astype(F8NP))
        # fp16 leftover taps pre-scaled by 32 (power of two: exact in fp16)
        # so the uniform 1/32 drain scale applies to the whole PSUM
        out[fkey] = np.ascontiguousarray((gl[:, k:] * WSC).astype(F16NP))
    out["wsuml"] = f16a(wsum)

    wse = np.zeros((128, 2, 2, 128), np.float32)
    for jj, (key, src) in enumerate((("se1l", "se1_w2"), ("se2l", "se2_w2"))):
        wsrc = inp[src]  # (256, 128, 3, 3)
        sl = np.zeros((128, 9, 2, 128), np.float32)
        for s9 in range(9):
            sy, sx = divmod(s9, 3)
            for hh in range(2):
                sl[:, s9, hh, :] = wsrc[hh * 128 + m, :, sy, sx].T
        wse[:, jj, :, :] = sl.sum(axis=1)
        out[key] = np.ascontiguousarray((sl * WSC).astype(F8NP))
    out["wssel"] = f16a(wse)

    for key, src in (("c1l", "conv1_w"), ("c3l", "conv3_w")):
        wsrc = inp[src][:, :, 0, 0]  # (256, 256) [cout, cin]
        cl = np.zeros((128, 2, 2, 128), np.float32)
        for kg in range(2):
            for hh in range(2):
                # cl[k, kg, hh, mo] = w[hh*128+mo, kg*128+k]
                cl[:, kg, hh, :] = wsrc[hh * 128:(hh + 1) * 128,
                                        kg * 128:(kg + 1) * 128].T
        out[key] = f16a(cl)
    return out


def _prep_bvec(inp, top_edge, bot_edge):
    bvec = np.zeros((128, 58), np.float32)
    for j, k in enumerate(("gw1_b1", "gw2_b1", "se1_b1", "se2_b1")):
        bvec[:, j] = inp[k]
    for base, k in ((4, "gw1_b2"), (22, "gw2_b2")):
        b2 = inp[k]
        for g in range(18):
            t, hh = g // 2, g % 2
            bvec[:, base + g] = b2[(hh * 128 + np.arange(128)) * 9 + t]
    for base, k in ((40, "se1_b2"), (42, "se2_b2"), (44, "conv1_b"), (46, "conv3_b"),
                    (48, "bn1_g"), (50, "bn1_b"), (52, "bn2_g"), (54, "bn2_b")):
        v = inp[k]
        bvec[:, base] = v[:128]
        bvec[:, base + 1] = v[128:]
    bvec[:, 56] = 0.0 if top_edge else 1.0
    bvec[:, 57] = 0.0 if bot_edge else 1.0
    return bvec


def _prep_shard(inp, i, wshared):
    n, blk = i // 4, i % 4
    s = RB * blk
    e = s + RB
    x = inp["x"][n]                       # (256,128,128)
    seg_ds = inp["seg"][n][:, ::2, ::2]   # (35,128,128)

    xg = x.reshape(2, 128, H, W)

    x_bf = np.zeros((128, 2, 36, 128), F16NP)
    lo, hi = max(s - 2, 0), min(e + 2, H)
    x_bf[:, :, lo - (s - 2):hi - (s - 2), :] = \
        xg[:, :, lo:hi, :].transpose(1, 0, 2, 3).astype(F16NP)

    seg3 = np.zeros((105, 36, 130), F16NP)
    seg_f = seg_ds.astype(F16NP)
    for sy in range(3):
        # seg3[sy*35+c, j, 1+x] = segmap[c, s-3+j+sy, x]
        r0 = s - 3 + sy
        lo, hi = max(r0, 0), min(r0 + 36, H)
        if hi > lo:
            seg3[sy * 35:(sy + 1) * 35, lo - r0:hi - r0, 1:129] = seg_f[:, lo:hi, :]

    return {
        "x_bf": x_bf,
        "seg3": np.ascontiguousarray(seg3),
        "bvec": _prep_bvec(inp, s == 0, e == H),
        **wshared,
    }


def kernel(**inputs):
    inp = {k: np.asarray(v) for k, v in inputs.items()}

    if "nc" not in _CACHE:
        _CACHE["nc"] = _build()
    nc = _CACHE["nc"]

    wshared = _prep_weights(inp)
    in_maps = [_prep_shard(inp, i, wshared) for i in range(N_CORES)]

    from concourse.bass_utils import run_bass_kernel_spmd
    res = run_bass_kernel_spmd(nc, in_maps, core_ids=list(range(N_CORES)),
                               trace=False)

    out = np.zeros((N, C, H, W), np.float32)
    for i in range(N_CORES):
        n, blk = i // 4, i % 4
        s = RB * blk
        o = res.results[i]["out"]  # (128, 2, 32, 128)
        out[n, :, s:s + RB, :] = o.transpose(1, 0, 2, 3).reshape(C, RB, W)
    return out
